# revision 18
# baseline (speedup 1.0000x reference)
"""Trainium2 Bass kernel for CRF negative log-likelihood loss (nn_CRF).

Sharding: data-parallel, 8 cores x 64 batch rows; per-core partial sums of
(logZ - gold score) are returned and summed/averaged on the host.

Normalizer (the sequential part): forward and backward CRF recurrences in
probability space meet at the sequence midpoint, halving the chain to 511
steps. Both chains are stacked in one [112, 64] tile (fwd states at
partitions 0-47, bwd at 64-111 so the final bwd-only matmul lands on a legal
PE tile boundary) and advance together: one bf16 matmul against a constant
block-diagonal [112,112] stationary + one DVE multiply with bulk-
pre-exponentiated emissions, exp(em - 4.9375) (constant centers the growth).
Range control: every 64 steps a per-column sum is taken with a ones-vector
matmul, inverted on DVE, broadcast with a K=1 matmul, and folded into the
emission slab 4 steps later - fully off the critical chain (the scale
commutes through the matmul). Log corrections accumulate via a deferred
Ln pass at the end.

Gold-path score: tag one-hots are built with packed bf16 is_equal ops into
an interleaved [em_j | oh_j] per-step layout so that the moving window
[oh_{j-1} | em_j] is a single contiguous 96-column slice.  One PE matmul per
timestep (stationary oh_j) then accumulates BOTH the transition-pair count
matrix (cols 0-47) and the gathered-emission matrix (cols 48-95) into one
[48,96] PSUM bank across all 512 steps; a tiny elementwise pass against
[transitions | I] finishes the score.  The matmuls are paced one per scan
step in program order so they hide inside the PE's idle window without
stretching the chain.  Start/end lookups are tiny one-hot matmul dots.

All partition-axis reductions are ones-vector matmuls (gpsimd C-reduce is
~5-13us per op on this path and partition_broadcast / indirect_copy /
tensor_tensor_reduce fail walrus codegen entirely).
"""
import os
import sys

import numpy as np
import ml_dtypes

for _p in ("/opt/trn_rl_repo", "/root/.axon_site/_ro/trn_rl_repo"):
    if os.path.isdir(_p) and _p not in sys.path:
        sys.path.insert(0, _p)

import concourse.bass as bass
import concourse.bacc as bacc
import concourse.mybir as mybir
import concourse.tile as tile

# Enable walrus's redundant-weight-load elision: the scan reuses one
# stationary for 511 matmuls and the default (disabled) reloads it each time.
if os.environ.get("LDW_OPT", "0") == "1":
    from concourse import bass_utils as _bu

    _orig_run_command = _bu.run_command

    def _run_command_ldw(argv, **kw):
        argv = ["--enable-ldw-opt=true" if a == "--enable-ldw-opt=false" else a
                for a in argv]
        return _orig_run_command(argv, **kw)

    _bu.run_command = _run_command_ldw

B, S, T = 512, 1024, 48
NCORES = 8
BL = B // NCORES  # 64
NSTEPS = (S - 2) // 2  # 511 paired fwd/bwd loop iterations
CBIAS = 4.9375  # constant folded into exp() of each step's emissions
RK = 64  # renorm cadence
NREN = (NSTEPS - 1) // RK  # 7 renorms at k = 64, 128, ..., 448
CHUNK = 64  # scan slabs per DMA/exp chunk (8 chunks of 64 slabs)
NC = 8  # numerator chunks (64 steps each)
NJ = 64  # steps per numerator chunk
NB = 96  # per-step block stride in the interleaved numerator layout
CW = 48 + NJ * NB  # comb tile width: leading oh_{-1} pad + 64 blocks
SL = 4  # steps per one-hot is_equal slice

BF16 = mybir.dt.bfloat16
F32 = mybir.dt.float32
AL = mybir.AluOpType
AX = mybir.AxisListType
AF = mybir.ActivationFunctionType

bf16np = ml_dtypes.bfloat16


def _build_graph():
    nc = bacc.Bacc("TRN2", target_bir_lowering=False, debug=False)

    emstack = nc.dram_tensor("emstack", [112, 512 * BL], BF16, kind="ExternalInput")
    slab0 = nc.dram_tensor("slab0", [96, BL], F32, kind="ExternalInput")
    bias96 = nc.dram_tensor("bias96", [96, 1], F32, kind="ExternalInput")
    transT = nc.dram_tensor("transT", [T, T], F32, kind="ExternalInput")
    transN = nc.dram_tensor("transN", [T, T], F32, kind="ExternalInput")
    emint = nc.dram_tensor("emint", [128, NC * CW], BF16, kind="ExternalInput")
    tagsnat = nc.dram_tensor("tagsnat", [BL, S], BF16, kind="ExternalInput")
    tpcrep = nc.dram_tensor("tpcrep", [128, 512 * T], BF16, kind="ExternalInput")
    tprev0 = nc.dram_tensor("tprev0", [128, NC], BF16, kind="ExternalInput")
    iotabd = nc.dram_tensor("iotabd", [128, T], BF16, kind="ExternalInput")
    iorepd = nc.dram_tensor("iorepd", [128, SL * T], BF16, kind="ExternalInput")
    numcoefd = nc.dram_tensor("numcoefd", [T, 2 * T], F32, kind="ExternalInput")
    startv = nc.dram_tensor("startv", [T, 1], F32, kind="ExternalInput")
    endv = nc.dram_tensor("endv", [T, 1], F32, kind="ExternalInput")
    outd = nc.dram_tensor("out", [1, 1], F32, kind="ExternalOutput")

    with tile.TileContext(nc) as tc:
        _kern(tc, nc, emstack, slab0, bias96, transT, transN, emint, tagsnat,
              tpcrep, tprev0, iotabd, iorepd, numcoefd, startv, endv, outd)
    nc.compile()
    return nc


def _kern(tc, nc, emstack, slab0, bias96, transT, transN, emint, tagsnat,
          tpcrep, tprev0, iotabd, iorepd, numcoefd, startv, endv, outd):
    from contextlib import ExitStack
    ctx = ExitStack()
    const = ctx.enter_context(tc.tile_pool(name="const", bufs=1))
    statep = ctx.enter_context(tc.tile_pool(name="state", bufs=4))
    psp = ctx.enter_context(tc.tile_pool(name="psp", bufs=3, space="PSUM"))
    psx = ctx.enter_context(tc.tile_pool(name="psx", bufs=1, space="PSUM"))
    psn = ctx.enter_context(tc.tile_pool(name="psn", bufs=1, space="PSUM"))
    rawp = ctx.enter_context(tc.tile_pool(name="raw", bufs=3))
    expdp = ctx.enter_context(tc.tile_pool(name="expd", bufs=2))
    combp = ctx.enter_context(tc.tile_pool(name="comb", bufs=2))
    trcp = ctx.enter_context(tc.tile_pool(name="trc", bufs=2))
    smallp = ctx.enter_context(tc.tile_pool(name="small", bufs=1))

    # ---------- constants / small inputs ----------
    bigm = const.tile([112, 112], BF16)
    trT = const.tile([T, T], F32)
    trS = const.tile([112, T], F32)
    biasT = const.tile([112, 1], F32)
    stS = const.tile([T, 1], F32)
    enS = const.tile([T, 1], F32)
    sl0 = const.tile([112, BL], F32)
    tagS = const.tile([BL, S], BF16)
    iotab = const.tile([128, T], BF16)
    iorep = const.tile([128, SL * T], BF16)
    tprev = const.tile([128, NC], BF16)
    numcoef = const.tile([T, 2 * T], F32)
    onescol = const.tile([112, 1], BF16)

    ones48 = const.tile([T, 1], BF16)
    ones64 = const.tile([BL, 1], BF16)
    onesrow = const.tile([1, 112], BF16)
    nc.vector.memset(onesrow[:], 1.0)
    mstore = const.tile([1, RK * 8], F32)
    cbias = const.tile([112, 1], F32)
    nc.vector.memset(cbias[:], -CBIAS)
    finc = const.tile([1, 1], F32)
    nc.vector.memset(finc[:], float(BL * 2 * NSTEPS * CBIAS))

    # DMA emission order == single-FIFO-queue service order: scan-critical
    # transfers first, numerator streams strictly after the early scan chunks.
    raw0 = rawp.tile([112, CHUNK * BL], BF16, tag="raw")
    nc.sync.dma_start(raw0[0:112, 0:8 * BL], emstack[:, 0:8 * BL])
    nc.vector.memset(biasT[:], 0.0)
    nc.vector.memset(sl0[:], 0.0)
    nc.sync.dma_start(biasT[0:T, :], bias96[0:T, :])
    nc.sync.dma_start(biasT[64:112, :], bias96[T:96, :])
    nc.sync.dma_start(sl0[0:T, :], slab0[0:T, :])
    nc.sync.dma_start(sl0[64:112, :], slab0[T:96, :])
    nc.sync.dma_start(trT[:], transT[:, :])
    nc.sync.dma_start(trS[64:112, :], transN[:, :])
    # warm the ACT Exp table while the first transfers are in flight; the Ln
    # table load is stamped into the scan body so it never gates startup
    actwarm = const.tile([1, 1], F32)
    nc.vector.memset(actwarm[:], 1.0)
    nc.scalar.activation(actwarm[:], actwarm[:], AF.Exp)
    with tc.tile_wait_until(12):
        nc.scalar.activation(actwarm[:], actwarm[:], AF.Ln)
    # scan chunks 1-2 ahead of any numerator bytes (rawp bufs=3 keeps the
    # FIFO head unblocked)
    raw_tiles = {0: raw0}
    for ci, (c0, clen) in ((1, (8, 16)), (2, (24, 32))):
        rw = rawp.tile([112, CHUNK * BL], BF16, tag="raw")
        nc.sync.dma_start(rw[0:112, 0:clen * BL],
                          emstack[:, c0 * BL:(c0 + clen) * BL])
        raw_tiles[ci] = rw
    # non-critical small inputs
    nc.sync.dma_start(stS[:], startv[:, :])
    nc.sync.dma_start(enS[:], endv[:, :])
    nc.sync.dma_start(iotab[:], iotabd[:, :])
    nc.sync.dma_start(iorep[:], iorepd[:, :])
    nc.sync.dma_start(tprev[:], tprev0[:, :])
    nc.sync.dma_start(numcoef[:], numcoefd[:, :])

    nc.vector.memset(onescol[:], 1.0)
    nc.vector.memset(ones48[:], 1.0)
    nc.vector.memset(ones64[:], 1.0)
    nc.vector.memset(mstore[:], 1.0)  # unused slots log to 0

    # blockdiag([exp(transT), exp(transN)]) in bf16; stamps pin the ACT order
    # (state init and bigm ahead of the chunk exps) so the first scan matmul's
    # cumulative ACT threshold stays minimal
    nc.vector.memset(bigm[:], 0.0)
    with tc.tile_wait_until(0.02):
        nc.scalar.activation(bigm[0:T, 0:T], trT[:], AF.Exp)
        nc.scalar.activation(bigm[64:112, 64:112], trS[64:112, :], AF.Exp)

    # ---------- state init: exp(slab0 + [start;end]) ----------
    state = statep.tile([112, BL], BF16, tag="state")
    with tc.tile_wait_until(0.01):
        nc.scalar.activation(state[:], sl0[:], AF.Exp, bias=biasT[:])

    # numerator chunk-0 streams (after the scan-critical DMAs)
    comb_tiles = {}
    trc_tiles = {}

    def num_chunk_dma(c):
        cb = combp.tile([128, CW], BF16, tag="comb")
        nc.sync.dma_start(cb[:], emint[:, c * CW:(c + 1) * CW])
        tr = trcp.tile([128, NJ * T], BF16, tag="trc")
        nc.sync.dma_start(tr[:], tpcrep[:, c * NJ * T:(c + 1) * NJ * T])
        comb_tiles[c] = cb
        trc_tiles[c] = tr

    num_chunk_dma(0)
    nc.sync.dma_start(tagS[:], tagsnat[:, :])

    psBig = psn.tile([T, 2 * T], F32, tag="psBig")

    def num_slice(s):
        """Build one-hots for steps [SL*s, SL*(s+1)); boundary col at s%16==0.

        Ordered BEFORE a scan mul in DVE program order so it executes inside
        the mul's semaphore-wait window and never extends the chain.
        """
        c = (SL * s) // NJ
        cb = comb_tiles[c]
        if (SL * s) % NJ == 0:
            nc.vector.tensor_tensor(
                cb[:, 0:T], iotab[:], tprev[:, c:c + 1].broadcast_to([128, T]),
                op=AL.is_equal)
        s0 = (SL * s) % NJ
        oh_view = cb[:, 48:].rearrange("p (j w) -> p j w", w=NB)
        nc.vector.tensor_tensor(
            oh_view[:, s0:s0 + SL, T:NB],
            iorep[:, 0:SL * T].rearrange("p (j w) -> p j w", w=T),
            trc_tiles[c][:, s0 * T:(s0 + SL) * T].rearrange(
                "p (j w) -> p j w", w=T),
            op=AL.is_equal)

    def num_mm(j):
        """Single count/emission matmul for step j (PE, after the scan mm)."""
        c, jj = j // NJ, j % NJ
        cb = comb_tiles[c]
        nc.tensor.matmul(psBig[:], cb[:, NB * jj + 96:NB * jj + 144],
                         cb[:, NB * jj:NB * jj + NB],
                         start=(j == 0), stop=(j == 511), skip_group_check=True)

    # ---------- main scan loop ----------
    # chunk boundaries: small leading chunks so the scan starts early
    bounds = [0, 8, 24, 56] + [56 + 64 * i for i in range(1, 8)] + [512]
    start_of = {}
    for ci in range(len(bounds) - 1):
        for p in range(bounds[ci], bounds[ci + 1]):
            start_of[p] = (ci, bounds[ci], bounds[ci + 1] - bounds[ci])
    expd_tiles = {}
    pend = None  # deferred renorm: (apply_at_k, bcastP)
    nren = 0
    CONS_LAG = 16  # scan steps between one-hot availability and its matmul
    PROD_LEAD = 8  # scan steps a one-hot slice is produced ahead of use
    mm_pending = []
    next_slice = 0
    next_mm = 0
    for k in range(1, NSTEPS + 1):
        ci, c0, clen = start_of[k - 1]
        if k - 1 == c0:
            if ci in raw_tiles:
                raw = raw_tiles[ci]
            else:
                raw = rawp.tile([112, CHUNK * BL], BF16, tag="raw")
                nc.sync.dma_start(raw[0:112, 0:clen * BL],
                                  emstack[:, c0 * BL:(c0 + clen) * BL])
            expd = expdp.tile([112, CHUNK * BL], BF16, tag="expd")
            with tc.tile_wait_until(0.1 + 0.01 * ci):
                nc.scalar.activation(expd[0:112, 0:clen * BL],
                                     raw[0:112, 0:clen * BL], AF.Exp,
                                     bias=cbias[:])
            expd_tiles[ci] = expd
        expd = expd_tiles[ci]
        j = (k - 1) - c0

        # numerator chunk DMA: emit when the previous chunk starts consuming
        # (unstamped: the scheduler hoists it as early as pool buffers allow)
        if (k - CONS_LAG - 1) % NJ == 0 and 1 <= (k - CONS_LAG - 1) // NJ + 1 < NC:
            num_chunk_dma((k - CONS_LAG - 1) // NJ + 1)

        # logical-time stamps pin the per-step interleave: the Tile scheduler
        # otherwise bunches all numerator matmuls into the earliest steps,
        # stretching the latency chain (measured +35us).
        with tc.tile_wait_until(k):
            ps = psp.tile([112, BL], F32, tag="ps")
            nc.tensor.matmul(ps[:], bigm[:], state[:], start=True, stop=True)

        # numerator matmuls ride the PE queue right behind the scan matmul
        if k % RK != 0:
            budget = 2
            while mm_pending and budget:
                with tc.tile_wait_until(k + 0.25):
                    num_mm(mm_pending.pop(0))
                budget -= 1
        if k - 1 - CONS_LAG >= 0 and next_mm <= k - 1 - CONS_LAG:
            mm_pending.append(next_mm)
            next_mm += 1

        # one-hot production on DVE, ordered BEFORE this step's scan mul
        if k - 1 - CONS_LAG + PROD_LEAD >= SL * next_slice and next_slice < 512 // SL:
            with tc.tile_wait_until(k + 0.4):
                num_slice(next_slice)
            next_slice += 1

        nstate = statep.tile([112, BL], BF16, tag="state")
        eop = expd[:, j * BL:(j + 1) * BL]
        if pend is not None and pend[0] == k:
            with tc.tile_wait_until(k + 0.45):
                esc = expdp.tile([112, BL], BF16, tag="esc")
                nc.vector.tensor_mul(esc[:], eop, pend[1][:])
            eop = esc[:]
            pend = None
        with tc.tile_wait_until(k + 0.5):
            nc.vector.tensor_mul(nstate[:], ps[:], eop)
        state = nstate

        if k % RK == 0 and k < NSTEPS:
            # off-chain: col-sum -> recip -> broadcast; applied at step k+4
            nren += 1
            with tc.tile_wait_until(k + 0.6):
                mxp = psx.tile([1, BL], F32, tag="mxp")
                nc.tensor.matmul(mxp[:], onescol[:], state[:], start=True, stop=True)
                mx = mstore[0:1, (nren - 1) * BL:nren * BL]
                nc.vector.tensor_copy(mx, mxp[:])
                rcp = smallp.tile([1, BL], BF16, tag="rcp")
                with nc.allow_low_precision(reason="renorm scale; log(mx) compensates"):
                    nc.vector.reciprocal(rcp[:], mxp[:])
                bcast = psx.tile([112, BL], F32, tag="bcp")
                nc.tensor.matmul(bcast[:], onesrow[:], rcp[:], start=True, stop=True)
            pend = (k + 4, bcast)

    # leftover numerator work (CONS_LAG tail) overlaps the final combine
    with tc.tile_wait_until(NSTEPS + 1):
        while next_slice < 512 // SL:
            num_slice(next_slice)
            next_slice += 1
        while next_mm <= 511:
            mm_pending.append(next_mm)
            next_mm += 1
        for j2 in mm_pending:
            num_mm(j2)

    # renorm-log pass: mstore is complete after the last renorm (k=448);
    # stamp it into the scan body so the tail doesn't pay the Ln + reduce
    with tc.tile_wait_until(RK * NREN + 8.5):
        lnm = smallp.tile([1, RK * 8], F32, tag="lnm")
        nc.scalar.activation(lnm[:], mstore[:], AF.Ln)
        carry = smallp.tile([1, BL], F32, tag="carry")
        nc.vector.tensor_reduce(
            carry[:], lnm[0:1, :].rearrange("p (j b) -> p b j", j=8), axis=AX.X,
            op=AL.add)

    # ---------- numerator: start/end lookups ----------
    ctx.enter_context(tc.tile_wait_until(NSTEPS + 2))

    def edge_dot(tag_col, vec, name):
        oh0 = smallp.tile([BL, T], BF16, tag=f"oh0{name}")
        i2 = iotab[0:BL, :]
        t2 = tag_col.broadcast_to([BL, T])
        nc.vector.tensor_tensor(oh0[:], i2, t2, op=AL.is_equal)
        cnt = psx.tile([T, 1], F32, tag="cnt")
        nc.tensor.matmul(cnt[:], oh0[:], ones64[:], start=True, stop=True)
        dots = smallp.tile([T, 1], BF16, tag=f"dots{name}")
        with nc.allow_low_precision(reason="scalar total; tolerant"):
            nc.vector.tensor_mul(dots[:], cnt[:], vec[:])
        ssump = psx.tile([1, 1], F32, tag="sum")
        nc.tensor.matmul(ssump[:], dots[:], ones48[:], start=True, stop=True)
        ssum = smallp.tile([1, 1], F32, tag=f"ssum{name}")
        nc.scalar.activation(ssum[:], ssump[:], AF.Copy)
        return ssum

    stsum = edge_dot(tagS[:, 0:1], stS, "st")
    ensum = edge_dot(tagS[:, S - 1:S], enS, "en")

    # ---------- combine fwd/bwd: Z = a_511 . (M @ g_512) ----------
    psf = psx.tile([T, BL], F32, tag="bcp")
    nc.tensor.matmul(psf[:], bigm[64:112, 64:112], state[64:112, :], start=True,
                     stop=True)
    stateF = smallp.tile([T, BL], F32, tag="stateF")
    nc.scalar.activation(stateF[:], state[0:T, :], AF.Copy)
    z1 = smallp.tile([T, BL], BF16, tag="z1")
    with nc.allow_low_precision(reason="z products; log tolerant"):
        nc.vector.tensor_mul(z1[:], stateF[:], psf[:])
    pz = psx.tile([1, BL], F32, tag="mxp")
    nc.tensor.matmul(pz[:], ones48[:], z1[:], start=True, stop=True)

    lz = smallp.tile([1, BL], F32, tag="lz")
    nc.scalar.activation(lz[:], pz[:], AF.Ln)
    # logZ = lz + 2*carry + 2*NSTEPS*CBIAS
    lzc = smallp.tile([1, BL], F32, tag="lzc")
    nc.vector.scalar_tensor_tensor(lzc[:], carry[:], 2.0, lz[:], op0=AL.mult, op1=AL.add)
    lzsum = smallp.tile([1, 1], F32, tag="lzsum")
    nc.vector.tensor_reduce(lzsum[:], lzc[:], axis=AX.X, op=AL.add)

    # ---------- numerator finalize: sum(psBig * [transitions | I]) ----------
    ct = smallp.tile([T, 2 * T], F32, tag="ct")
    nc.vector.tensor_mul(ct[:], psBig[:], numcoef[:])
    ctr = smallp.tile([T, 1], F32, tag="ctr")
    nc.vector.tensor_reduce(ctr[:], ct[:], axis=AX.X, op=AL.add)
    ctrb = smallp.tile([T, 1], BF16, tag="ctrb")
    with nc.allow_low_precision(reason="scalar total; tolerant"):
        nc.vector.tensor_copy(ctrb[:], ctr[:])
    ctsump = psx.tile([1, 1], F32, tag="sum")
    nc.tensor.matmul(ctsump[:], ctrb[:], ones48[:], start=True, stop=True)
    ctsum = smallp.tile([1, 1], F32, tag="ctsum")
    nc.scalar.activation(ctsum[:], ctsump[:], AF.Copy)

    # ---------- total = lzsum + finc - ctsum - stsum - ensum ----------
    t1 = smallp.tile([1, 1], F32, tag="t1")
    nc.vector.scalar_tensor_tensor(t1[:], lzsum[:], finc[:], ctsum[:],
                                   op0=AL.add, op1=AL.subtract)
    t2 = smallp.tile([1, 1], F32, tag="t2")
    nc.vector.scalar_tensor_tensor(t2[:], t1[:], stsum[:], ensum[:],
                                   op0=AL.subtract, op1=AL.subtract)
    nc.sync.dma_start(outd[:, :], t2[:])
    ctx.close()


def _prep_core_inputs(em, tags, transitions, start, end):
    """em [BL,S,T] f32, tags [BL,S] int64 -> dict of device arrays."""
    em = np.asarray(em, dtype=np.float32)
    tags = np.asarray(tags).astype(np.int32)

    # emstack [112, 512*BL]: pos j holds slab (j+1): upper em[:,j+1,:]^T,
    # lower em[:,1022-j,:]^T; pos 511 is padding.
    emstack = np.zeros((112, 512, BL), dtype=np.float32)
    emstack[0:T, 0:NSTEPS] = em[:, 1:NSTEPS + 1, :].transpose(2, 1, 0)
    emstack[64:112, 0:NSTEPS] = em[:, S - 2:S - 2 - NSTEPS:-1, :].transpose(2, 1, 0)
    emstack = emstack.reshape(112, 512 * BL).astype(bf16np)

    slab0 = np.concatenate([em[:, 0, :].T, em[:, S - 1, :].T], axis=0).astype(np.float32)
    bias96 = np.concatenate([start, end])[:, None].astype(np.float32)

    # interleaved numerator emissions: per chunk c, 48-col pad then 64 blocks
    # of [em_j (48) | oh_j slot (48, zero)]
    emint = np.zeros((BL * 2, NC, CW), dtype=bf16np)
    emr = em.reshape(BL, 2, NC, NJ, T).reshape(BL * 2, NC, NJ, T)
    emint[:, :, 48:].reshape(BL * 2, NC, NJ, NB)[:, :, :, 0:T] = emr
    emint = emint.reshape(128, NC * CW)

    tpcur = tags.reshape(BL, 2, 512).reshape(BL * 2, 512)
    # prev-tag value for step j=0 of each chunk (sentinel T for (h=0, c=0))
    tprev0 = np.zeros((BL * 2, NC), dtype=np.int32)
    for c in range(NC):
        if c == 0:
            tprev0[0::2, 0] = T
            tprev0[1::2, 0] = tags[:, 511]
        else:
            tprev0[:, c] = tpcur[:, c * NJ - 1]

    iotab = np.tile(np.arange(T, dtype=np.float32), (128, 1))
    iorep = np.tile(np.arange(T, dtype=np.float32), (128, SL))
    numcoef = np.concatenate(
        [np.asarray(transitions, dtype=np.float32), np.eye(T, dtype=np.float32)],
        axis=1)

    return {
        "emstack": emstack,
        "slab0": slab0,
        "bias96": bias96,
        "transT": np.ascontiguousarray(transitions.T).astype(np.float32),
        "transN": np.ascontiguousarray(transitions).astype(np.float32),
        "emint": emint,
        "tagsnat": tags.astype(bf16np),
        "tpcrep": np.repeat(tpcur.astype(bf16np), T).reshape(128, 512 * T),
        "tprev0": tprev0.astype(bf16np),
        "iotabd": iotab.astype(bf16np),
        "iorepd": iorep.astype(bf16np),
        "numcoefd": numcoef,
        "startv": start[:, None].astype(np.float32),
        "endv": end[:, None].astype(np.float32),
    }


def prep_all_inputs(emissions, tags, mask, transitions, start_transitions,
                    end_transitions):
    em = np.asarray(emissions, dtype=np.float32)
    tg = np.asarray(tags)
    tr = np.asarray(transitions, dtype=np.float32)
    st = np.asarray(start_transitions, dtype=np.float32)
    en = np.asarray(end_transitions, dtype=np.float32)
    return [
        _prep_core_inputs(em[c * BL:(c + 1) * BL], tg[c * BL:(c + 1) * BL], tr, st, en)
        for c in range(NCORES)
    ]


_NC_CACHE = {}


def get_graph():
    if "nc" not in _NC_CACHE:
        _NC_CACHE["nc"] = _build_graph()
    return _NC_CACHE["nc"]


def kernel(emissions, tags, mask, transitions, start_transitions, end_transitions,
           **kw):
    from concourse import bass_utils
    nc = get_graph()
    in_maps = prep_all_inputs(emissions, tags, mask, transitions,
                              start_transitions, end_transitions)
    res = bass_utils.run_bass_kernel_spmd(nc, in_maps, core_ids=list(range(NCORES)))
    total = sum(float(res.results[c]["out"][0, 0]) for c in range(NCORES))
    return np.float32(total / B)


if __name__ == "__main__":
    get_graph()
    print("graph built ok")


# revision 19
# speedup vs baseline: 1.0071x; 1.0071x over previous
"""Trainium2 Bass kernel for CRF negative log-likelihood loss (nn_CRF).

Sharding: data-parallel, 8 cores x 64 batch rows; per-core partial sums of
(logZ - gold score) are returned and summed/averaged on the host.

Normalizer (the sequential part): forward and backward CRF recurrences in
probability space meet at the sequence midpoint, halving the chain to 511
steps. Both chains are stacked in one [112, 64] tile (fwd states at
partitions 0-47, bwd at 64-111 so the final bwd-only matmul lands on a legal
PE tile boundary) and advance together: one bf16 matmul against a constant
block-diagonal [112,112] stationary + one DVE multiply with bulk-
pre-exponentiated emissions, exp(em - 4.9375) (constant centers the growth).
Range control: every 64 steps a per-column sum is taken with a ones-vector
matmul, inverted on DVE, broadcast with a K=1 matmul, and folded into the
emission slab 4 steps later - fully off the critical chain (the scale
commutes through the matmul). Log corrections accumulate via a deferred
Ln pass at the end.

Gold-path score: tag one-hots are built with packed bf16 is_equal ops into
an interleaved [em_j | oh_j] per-step layout so that the moving window
[oh_{j-1} | em_j] is a single contiguous 96-column slice.  One PE matmul per
timestep (stationary oh_j) then accumulates BOTH the transition-pair count
matrix (cols 0-47) and the gathered-emission matrix (cols 48-95) into one
[48,96] PSUM bank across all 512 steps; a tiny elementwise pass against
[transitions | I] finishes the score.  The matmuls are paced one per scan
step in program order so they hide inside the PE's idle window without
stretching the chain.  Start/end lookups are tiny one-hot matmul dots.

All partition-axis reductions are ones-vector matmuls (gpsimd C-reduce is
~5-13us per op on this path and partition_broadcast / indirect_copy /
tensor_tensor_reduce fail walrus codegen entirely).
"""
import os
import sys

import numpy as np
import ml_dtypes

for _p in ("/opt/trn_rl_repo", "/root/.axon_site/_ro/trn_rl_repo"):
    if os.path.isdir(_p) and _p not in sys.path:
        sys.path.insert(0, _p)

import concourse.bass as bass
import concourse.bacc as bacc
import concourse.mybir as mybir
import concourse.tile as tile

# Enable walrus's redundant-weight-load elision: the scan reuses one
# stationary for 511 matmuls and the default (disabled) reloads it each time.
if os.environ.get("LDW_OPT", "0") == "1":
    from concourse import bass_utils as _bu

    _orig_run_command = _bu.run_command

    def _run_command_ldw(argv, **kw):
        argv = ["--enable-ldw-opt=true" if a == "--enable-ldw-opt=false" else a
                for a in argv]
        return _orig_run_command(argv, **kw)

    _bu.run_command = _run_command_ldw

B, S, T = 512, 1024, 48
NCORES = 8
BL = B // NCORES  # 64
NSTEPS = (S - 2) // 2  # 511 paired fwd/bwd loop iterations
CBIAS = 4.9375  # constant folded into exp() of each step's emissions
RK = 64  # renorm cadence
NREN = (NSTEPS - 1) // RK  # 7 renorms at k = 64, 128, ..., 448
CHUNK = 64  # scan slabs per DMA/exp chunk (8 chunks of 64 slabs)
NC = 8  # numerator chunks (64 steps each)
NJ = 64  # steps per numerator chunk
NB = 96  # per-step block stride in the interleaved numerator layout
CW = 48 + NJ * NB  # comb tile width: leading oh_{-1} pad + 64 blocks
SL = 4  # steps per one-hot is_equal slice

BF16 = mybir.dt.bfloat16
F32 = mybir.dt.float32
AL = mybir.AluOpType
AX = mybir.AxisListType
AF = mybir.ActivationFunctionType

bf16np = ml_dtypes.bfloat16


def _build_graph():
    nc = bacc.Bacc("TRN2", target_bir_lowering=False, debug=False)

    emstack = nc.dram_tensor("emstack", [112, 512 * BL], BF16, kind="ExternalInput")
    slab0 = nc.dram_tensor("slab0", [96, BL], F32, kind="ExternalInput")
    bias96 = nc.dram_tensor("bias96", [96, 1], F32, kind="ExternalInput")
    transT = nc.dram_tensor("transT", [T, T], F32, kind="ExternalInput")
    transN = nc.dram_tensor("transN", [T, T], F32, kind="ExternalInput")
    emint = nc.dram_tensor("emint", [128, NC * CW], BF16, kind="ExternalInput")
    tagsnat = nc.dram_tensor("tagsnat", [BL, S], BF16, kind="ExternalInput")
    tpcrep = nc.dram_tensor("tpcrep", [128, 512 * T], BF16, kind="ExternalInput")
    tprev0 = nc.dram_tensor("tprev0", [128, NC], BF16, kind="ExternalInput")
    iotabd = nc.dram_tensor("iotabd", [128, T], BF16, kind="ExternalInput")
    iorepd = nc.dram_tensor("iorepd", [128, SL * T], BF16, kind="ExternalInput")
    numcoefd = nc.dram_tensor("numcoefd", [T, 2 * T], F32, kind="ExternalInput")
    startv = nc.dram_tensor("startv", [T, 1], F32, kind="ExternalInput")
    endv = nc.dram_tensor("endv", [T, 1], F32, kind="ExternalInput")
    outd = nc.dram_tensor("out", [1, 1], F32, kind="ExternalOutput")

    with tile.TileContext(nc) as tc:
        _kern(tc, nc, emstack, slab0, bias96, transT, transN, emint, tagsnat,
              tpcrep, tprev0, iotabd, iorepd, numcoefd, startv, endv, outd)
    nc.compile()
    return nc


def _kern(tc, nc, emstack, slab0, bias96, transT, transN, emint, tagsnat,
          tpcrep, tprev0, iotabd, iorepd, numcoefd, startv, endv, outd):
    from contextlib import ExitStack
    ctx = ExitStack()
    const = ctx.enter_context(tc.tile_pool(name="const", bufs=1))
    statep = ctx.enter_context(tc.tile_pool(name="state", bufs=4))
    psp = ctx.enter_context(tc.tile_pool(name="psp", bufs=3, space="PSUM"))
    psx = ctx.enter_context(tc.tile_pool(name="psx", bufs=1, space="PSUM"))
    psn = ctx.enter_context(tc.tile_pool(name="psn", bufs=1, space="PSUM"))
    rawp = ctx.enter_context(tc.tile_pool(name="raw", bufs=3))
    expdp = ctx.enter_context(tc.tile_pool(name="expd", bufs=2))
    combp = ctx.enter_context(tc.tile_pool(name="comb", bufs=2))
    trcp = ctx.enter_context(tc.tile_pool(name="trc", bufs=2))
    smallp = ctx.enter_context(tc.tile_pool(name="small", bufs=1))

    # ---------- constants / small inputs ----------
    bigm = const.tile([112, 112], BF16)
    trT = const.tile([T, T], F32)
    trS = const.tile([112, T], F32)
    biasT = const.tile([112, 1], F32)
    stS = const.tile([T, 1], F32)
    enS = const.tile([T, 1], F32)
    sl0 = const.tile([112, BL], F32)
    tagS = const.tile([BL, S], BF16)
    iotab = const.tile([128, T], BF16)
    iorep = const.tile([128, SL * T], BF16)
    tprev = const.tile([128, NC], BF16)
    numcoef = const.tile([T, 2 * T], F32)
    onescol = const.tile([112, 1], BF16)

    ones48 = const.tile([T, 1], BF16)
    ones64 = const.tile([BL, 1], BF16)
    onesrow = const.tile([1, 112], BF16)
    nc.vector.memset(onesrow[:], 1.0)
    mstore = const.tile([1, RK * 8], F32)
    cbias = const.tile([112, 1], F32)
    nc.vector.memset(cbias[:], -CBIAS)
    finc = const.tile([1, 1], F32)
    nc.vector.memset(finc[:], float(BL * 2 * NSTEPS * CBIAS))

    # DMA emission order == single-FIFO-queue service order: scan-critical
    # transfers first, numerator streams strictly after the early scan chunks.
    raw0 = rawp.tile([112, CHUNK * BL], BF16, tag="raw")
    nc.sync.dma_start(raw0[0:112, 0:8 * BL], emstack[:, 0:8 * BL])
    nc.vector.memset(biasT[:], 0.0)
    nc.vector.memset(sl0[:], 0.0)
    nc.sync.dma_start(biasT[0:T, :], bias96[0:T, :])
    nc.sync.dma_start(biasT[64:112, :], bias96[T:96, :])
    nc.sync.dma_start(sl0[0:T, :], slab0[0:T, :])
    nc.sync.dma_start(sl0[64:112, :], slab0[T:96, :])
    nc.sync.dma_start(trT[:], transT[:, :])
    nc.sync.dma_start(trS[64:112, :], transN[:, :])
    # warm the ACT Exp table while the first transfers are in flight; the Ln
    # table load is stamped into the scan body so it never gates startup
    actwarm = const.tile([1, 1], F32)
    nc.vector.memset(actwarm[:], 1.0)
    nc.scalar.activation(actwarm[:], actwarm[:], AF.Exp)
    with tc.tile_wait_until(12):
        nc.scalar.activation(actwarm[:], actwarm[:], AF.Ln)
    # scan chunks 1-2 ahead of any numerator bytes (rawp bufs=3 keeps the
    # FIFO head unblocked)
    raw_tiles = {0: raw0}
    for ci, (c0, clen) in ((1, (8, 16)), (2, (24, 32))):
        rw = rawp.tile([112, CHUNK * BL], BF16, tag="raw")
        nc.sync.dma_start(rw[0:112, 0:clen * BL],
                          emstack[:, c0 * BL:(c0 + clen) * BL])
        raw_tiles[ci] = rw
    # non-critical small inputs
    nc.gpsimd.dma_start(stS[:], startv[:, :])
    nc.gpsimd.dma_start(enS[:], endv[:, :])
    nc.gpsimd.dma_start(iotab[:], iotabd[:, :])
    nc.gpsimd.dma_start(iorep[:], iorepd[:, :])
    nc.gpsimd.dma_start(tprev[:], tprev0[:, :])
    nc.gpsimd.dma_start(numcoef[:], numcoefd[:, :])

    nc.vector.memset(onescol[:], 1.0)
    nc.vector.memset(ones48[:], 1.0)
    nc.vector.memset(ones64[:], 1.0)
    nc.vector.memset(mstore[:], 1.0)  # unused slots log to 0

    # blockdiag([exp(transT), exp(transN)]) in bf16; stamps pin the ACT order
    # (state init and bigm ahead of the chunk exps) so the first scan matmul's
    # cumulative ACT threshold stays minimal
    nc.vector.memset(bigm[:], 0.0)
    with tc.tile_wait_until(0.02):
        nc.scalar.activation(bigm[0:T, 0:T], trT[:], AF.Exp)
        nc.scalar.activation(bigm[64:112, 64:112], trS[64:112, :], AF.Exp)

    # ---------- state init: exp(slab0 + [start;end]) ----------
    state = statep.tile([112, BL], BF16, tag="state")
    with tc.tile_wait_until(0.01):
        nc.scalar.activation(state[:], sl0[:], AF.Exp, bias=biasT[:])

    # numerator chunk-0 streams (after the scan-critical DMAs)
    comb_tiles = {}
    trc_tiles = {}

    def num_chunk_dma(c):
        cb = combp.tile([128, CW], BF16, tag="comb")
        nc.gpsimd.dma_start(cb[:], emint[:, c * CW:(c + 1) * CW])
        tr = trcp.tile([128, NJ * T], BF16, tag="trc")
        nc.gpsimd.dma_start(tr[:], tpcrep[:, c * NJ * T:(c + 1) * NJ * T])
        comb_tiles[c] = cb
        trc_tiles[c] = tr

    num_chunk_dma(0)
    nc.gpsimd.dma_start(tagS[:], tagsnat[:, :])

    psBig = psn.tile([T, 2 * T], F32, tag="psBig")

    def num_slice(s):
        """Build one-hots for steps [SL*s, SL*(s+1)); boundary col at s%16==0.

        Ordered BEFORE a scan mul in DVE program order so it executes inside
        the mul's semaphore-wait window and never extends the chain.
        """
        c = (SL * s) // NJ
        cb = comb_tiles[c]
        if (SL * s) % NJ == 0:
            nc.vector.tensor_tensor(
                cb[:, 0:T], iotab[:], tprev[:, c:c + 1].broadcast_to([128, T]),
                op=AL.is_equal)
        s0 = (SL * s) % NJ
        oh_view = cb[:, 48:].rearrange("p (j w) -> p j w", w=NB)
        nc.vector.tensor_tensor(
            oh_view[:, s0:s0 + SL, T:NB],
            iorep[:, 0:SL * T].rearrange("p (j w) -> p j w", w=T),
            trc_tiles[c][:, s0 * T:(s0 + SL) * T].rearrange(
                "p (j w) -> p j w", w=T),
            op=AL.is_equal)

    def num_mm(j):
        """Single count/emission matmul for step j (PE, after the scan mm)."""
        c, jj = j // NJ, j % NJ
        cb = comb_tiles[c]
        nc.tensor.matmul(psBig[:], cb[:, NB * jj + 96:NB * jj + 144],
                         cb[:, NB * jj:NB * jj + NB],
                         start=(j == 0), stop=(j == 511), skip_group_check=True)

    # ---------- main scan loop ----------
    # chunk boundaries: small leading chunks so the scan starts early
    bounds = [0, 8, 24, 56] + [56 + 64 * i for i in range(1, 8)] + [512]
    start_of = {}
    for ci in range(len(bounds) - 1):
        for p in range(bounds[ci], bounds[ci + 1]):
            start_of[p] = (ci, bounds[ci], bounds[ci + 1] - bounds[ci])
    expd_tiles = {}
    pend = None  # deferred renorm: (apply_at_k, bcastP)
    nren = 0
    CONS_LAG = 16  # scan steps between one-hot availability and its matmul
    PROD_LEAD = 8  # scan steps a one-hot slice is produced ahead of use
    mm_pending = []
    next_slice = 0
    next_mm = 0
    for k in range(1, NSTEPS + 1):
        ci, c0, clen = start_of[k - 1]
        if k - 1 == c0:
            if ci in raw_tiles:
                raw = raw_tiles[ci]
            else:
                raw = rawp.tile([112, CHUNK * BL], BF16, tag="raw")
                nc.sync.dma_start(raw[0:112, 0:clen * BL],
                                  emstack[:, c0 * BL:(c0 + clen) * BL])
            expd = expdp.tile([112, CHUNK * BL], BF16, tag="expd")
            with tc.tile_wait_until(0.1 + 0.01 * ci):
                nc.scalar.activation(expd[0:112, 0:clen * BL],
                                     raw[0:112, 0:clen * BL], AF.Exp,
                                     bias=cbias[:])
            expd_tiles[ci] = expd
        expd = expd_tiles[ci]
        j = (k - 1) - c0

        # numerator chunk DMA: emit when the previous chunk starts consuming
        # (unstamped: the scheduler hoists it as early as pool buffers allow)
        if (k - CONS_LAG - 1) % NJ == 0 and 1 <= (k - CONS_LAG - 1) // NJ + 1 < NC:
            num_chunk_dma((k - CONS_LAG - 1) // NJ + 1)

        # logical-time stamps pin the per-step interleave: the Tile scheduler
        # otherwise bunches all numerator matmuls into the earliest steps,
        # stretching the latency chain (measured +35us).
        with tc.tile_wait_until(k):
            ps = psp.tile([112, BL], F32, tag="ps")
            nc.tensor.matmul(ps[:], bigm[:], state[:], start=True, stop=True)

        # numerator matmuls ride the PE queue right behind the scan matmul
        if k % RK != 0:
            budget = 2
            while mm_pending and budget:
                with tc.tile_wait_until(k + 0.25):
                    num_mm(mm_pending.pop(0))
                budget -= 1
        if k - 1 - CONS_LAG >= 0 and next_mm <= k - 1 - CONS_LAG:
            mm_pending.append(next_mm)
            next_mm += 1

        # one-hot production on DVE, ordered BEFORE this step's scan mul
        if k - 1 - CONS_LAG + PROD_LEAD >= SL * next_slice and next_slice < 512 // SL:
            with tc.tile_wait_until(k + 0.4):
                num_slice(next_slice)
            next_slice += 1

        nstate = statep.tile([112, BL], BF16, tag="state")
        eop = expd[:, j * BL:(j + 1) * BL]
        if pend is not None and pend[0] == k:
            with tc.tile_wait_until(k + 0.45):
                esc = expdp.tile([112, BL], BF16, tag="esc")
                nc.vector.tensor_mul(esc[:], eop, pend[1][:])
            eop = esc[:]
            pend = None
        with tc.tile_wait_until(k + 0.5):
            nc.vector.tensor_mul(nstate[:], ps[:], eop)
        state = nstate

        if k % RK == 0 and k < NSTEPS:
            # off-chain: col-sum -> recip -> broadcast; applied at step k+4
            nren += 1
            with tc.tile_wait_until(k + 0.6):
                mxp = psx.tile([1, BL], F32, tag="mxp")
                nc.tensor.matmul(mxp[:], onescol[:], state[:], start=True, stop=True)
                mx = mstore[0:1, (nren - 1) * BL:nren * BL]
                nc.vector.tensor_copy(mx, mxp[:])
                rcp = smallp.tile([1, BL], BF16, tag="rcp")
                with nc.allow_low_precision(reason="renorm scale; log(mx) compensates"):
                    nc.vector.reciprocal(rcp[:], mxp[:])
                bcast = psx.tile([112, BL], F32, tag="bcp")
                nc.tensor.matmul(bcast[:], onesrow[:], rcp[:], start=True, stop=True)
            pend = (k + 4, bcast)

    # leftover numerator work (CONS_LAG tail) overlaps the final combine
    with tc.tile_wait_until(NSTEPS + 1):
        while next_slice < 512 // SL:
            num_slice(next_slice)
            next_slice += 1
        while next_mm <= 511:
            mm_pending.append(next_mm)
            next_mm += 1
        for j2 in mm_pending:
            num_mm(j2)

    # renorm-log pass: mstore is complete after the last renorm (k=448);
    # stamp it into the scan body so the tail doesn't pay the Ln + reduce
    with tc.tile_wait_until(RK * NREN + 8.5):
        lnm = smallp.tile([1, RK * 8], F32, tag="lnm")
        nc.scalar.activation(lnm[:], mstore[:], AF.Ln)
        carry = smallp.tile([1, BL], F32, tag="carry")
        nc.vector.tensor_reduce(
            carry[:], lnm[0:1, :].rearrange("p (j b) -> p b j", j=8), axis=AX.X,
            op=AL.add)

    # ---------- numerator: start/end lookups ----------
    ctx.enter_context(tc.tile_wait_until(NSTEPS + 2))

    def edge_dot(tag_col, vec, name):
        oh0 = smallp.tile([BL, T], BF16, tag=f"oh0{name}")
        i2 = iotab[0:BL, :]
        t2 = tag_col.broadcast_to([BL, T])
        nc.vector.tensor_tensor(oh0[:], i2, t2, op=AL.is_equal)
        cnt = psx.tile([T, 1], F32, tag="cnt")
        nc.tensor.matmul(cnt[:], oh0[:], ones64[:], start=True, stop=True)
        dots = smallp.tile([T, 1], BF16, tag=f"dots{name}")
        with nc.allow_low_precision(reason="scalar total; tolerant"):
            nc.vector.tensor_mul(dots[:], cnt[:], vec[:])
        ssump = psx.tile([1, 1], F32, tag="sum")
        nc.tensor.matmul(ssump[:], dots[:], ones48[:], start=True, stop=True)
        ssum = smallp.tile([1, 1], F32, tag=f"ssum{name}")
        nc.scalar.activation(ssum[:], ssump[:], AF.Copy)
        return ssum

    stsum = edge_dot(tagS[:, 0:1], stS, "st")
    ensum = edge_dot(tagS[:, S - 1:S], enS, "en")

    # ---------- combine fwd/bwd: Z = a_511 . (M @ g_512) ----------
    psf = psx.tile([T, BL], F32, tag="bcp")
    nc.tensor.matmul(psf[:], bigm[64:112, 64:112], state[64:112, :], start=True,
                     stop=True)
    stateF = smallp.tile([T, BL], F32, tag="stateF")
    nc.scalar.activation(stateF[:], state[0:T, :], AF.Copy)
    z1 = smallp.tile([T, BL], BF16, tag="z1")
    with nc.allow_low_precision(reason="z products; log tolerant"):
        nc.vector.tensor_mul(z1[:], stateF[:], psf[:])
    pz = psx.tile([1, BL], F32, tag="mxp")
    nc.tensor.matmul(pz[:], ones48[:], z1[:], start=True, stop=True)

    lz = smallp.tile([1, BL], F32, tag="lz")
    nc.scalar.activation(lz[:], pz[:], AF.Ln)
    # logZ = lz + 2*carry + 2*NSTEPS*CBIAS
    lzc = smallp.tile([1, BL], F32, tag="lzc")
    nc.vector.scalar_tensor_tensor(lzc[:], carry[:], 2.0, lz[:], op0=AL.mult, op1=AL.add)
    lzsum = smallp.tile([1, 1], F32, tag="lzsum")
    nc.vector.tensor_reduce(lzsum[:], lzc[:], axis=AX.X, op=AL.add)

    # ---------- numerator finalize: sum(psBig * [transitions | I]) ----------
    ct = smallp.tile([T, 2 * T], F32, tag="ct")
    nc.vector.tensor_mul(ct[:], psBig[:], numcoef[:])
    ctr = smallp.tile([T, 1], F32, tag="ctr")
    nc.vector.tensor_reduce(ctr[:], ct[:], axis=AX.X, op=AL.add)
    ctrb = smallp.tile([T, 1], BF16, tag="ctrb")
    with nc.allow_low_precision(reason="scalar total; tolerant"):
        nc.vector.tensor_copy(ctrb[:], ctr[:])
    ctsump = psx.tile([1, 1], F32, tag="sum")
    nc.tensor.matmul(ctsump[:], ctrb[:], ones48[:], start=True, stop=True)
    ctsum = smallp.tile([1, 1], F32, tag="ctsum")
    nc.scalar.activation(ctsum[:], ctsump[:], AF.Copy)

    # ---------- total = lzsum + finc - ctsum - stsum - ensum ----------
    t1 = smallp.tile([1, 1], F32, tag="t1")
    nc.vector.scalar_tensor_tensor(t1[:], lzsum[:], finc[:], ctsum[:],
                                   op0=AL.add, op1=AL.subtract)
    t2 = smallp.tile([1, 1], F32, tag="t2")
    nc.vector.scalar_tensor_tensor(t2[:], t1[:], stsum[:], ensum[:],
                                   op0=AL.subtract, op1=AL.subtract)
    nc.sync.dma_start(outd[:, :], t2[:])
    ctx.close()


def _prep_core_inputs(em, tags, transitions, start, end):
    """em [BL,S,T] f32, tags [BL,S] int64 -> dict of device arrays."""
    em = np.asarray(em, dtype=np.float32)
    tags = np.asarray(tags).astype(np.int32)

    # emstack [112, 512*BL]: pos j holds slab (j+1): upper em[:,j+1,:]^T,
    # lower em[:,1022-j,:]^T; pos 511 is padding.
    emstack = np.zeros((112, 512, BL), dtype=np.float32)
    emstack[0:T, 0:NSTEPS] = em[:, 1:NSTEPS + 1, :].transpose(2, 1, 0)
    emstack[64:112, 0:NSTEPS] = em[:, S - 2:S - 2 - NSTEPS:-1, :].transpose(2, 1, 0)
    emstack = emstack.reshape(112, 512 * BL).astype(bf16np)

    slab0 = np.concatenate([em[:, 0, :].T, em[:, S - 1, :].T], axis=0).astype(np.float32)
    bias96 = np.concatenate([start, end])[:, None].astype(np.float32)

    # interleaved numerator emissions: per chunk c, 48-col pad then 64 blocks
    # of [em_j (48) | oh_j slot (48, zero)]
    emint = np.zeros((BL * 2, NC, CW), dtype=bf16np)
    emr = em.reshape(BL, 2, NC, NJ, T).reshape(BL * 2, NC, NJ, T)
    emint[:, :, 48:].reshape(BL * 2, NC, NJ, NB)[:, :, :, 0:T] = emr
    emint = emint.reshape(128, NC * CW)

    tpcur = tags.reshape(BL, 2, 512).reshape(BL * 2, 512)
    # prev-tag value for step j=0 of each chunk (sentinel T for (h=0, c=0))
    tprev0 = np.zeros((BL * 2, NC), dtype=np.int32)
    for c in range(NC):
        if c == 0:
            tprev0[0::2, 0] = T
            tprev0[1::2, 0] = tags[:, 511]
        else:
            tprev0[:, c] = tpcur[:, c * NJ - 1]

    iotab = np.tile(np.arange(T, dtype=np.float32), (128, 1))
    iorep = np.tile(np.arange(T, dtype=np.float32), (128, SL))
    numcoef = np.concatenate(
        [np.asarray(transitions, dtype=np.float32), np.eye(T, dtype=np.float32)],
        axis=1)

    return {
        "emstack": emstack,
        "slab0": slab0,
        "bias96": bias96,
        "transT": np.ascontiguousarray(transitions.T).astype(np.float32),
        "transN": np.ascontiguousarray(transitions).astype(np.float32),
        "emint": emint,
        "tagsnat": tags.astype(bf16np),
        "tpcrep": np.repeat(tpcur.astype(bf16np), T).reshape(128, 512 * T),
        "tprev0": tprev0.astype(bf16np),
        "iotabd": iotab.astype(bf16np),
        "iorepd": iorep.astype(bf16np),
        "numcoefd": numcoef,
        "startv": start[:, None].astype(np.float32),
        "endv": end[:, None].astype(np.float32),
    }


def prep_all_inputs(emissions, tags, mask, transitions, start_transitions,
                    end_transitions):
    em = np.asarray(emissions, dtype=np.float32)
    tg = np.asarray(tags)
    tr = np.asarray(transitions, dtype=np.float32)
    st = np.asarray(start_transitions, dtype=np.float32)
    en = np.asarray(end_transitions, dtype=np.float32)
    return [
        _prep_core_inputs(em[c * BL:(c + 1) * BL], tg[c * BL:(c + 1) * BL], tr, st, en)
        for c in range(NCORES)
    ]


_NC_CACHE = {}


def get_graph():
    if "nc" not in _NC_CACHE:
        _NC_CACHE["nc"] = _build_graph()
    return _NC_CACHE["nc"]


def kernel(emissions, tags, mask, transitions, start_transitions, end_transitions,
           **kw):
    from concourse import bass_utils
    nc = get_graph()
    in_maps = prep_all_inputs(emissions, tags, mask, transitions,
                              start_transitions, end_transitions)
    res = bass_utils.run_bass_kernel_spmd(nc, in_maps, core_ids=list(range(NCORES)))
    total = sum(float(res.results[c]["out"][0, 0]) for c in range(NCORES))
    return np.float32(total / B)


if __name__ == "__main__":
    get_graph()
    print("graph built ok")


# revision 24
# speedup vs baseline: 1.0333x; 1.0261x over previous
"""Trainium2 Bass kernel for CRF negative log-likelihood loss (nn_CRF).

Sharding: data-parallel, 8 cores x 64 batch rows; per-core partial sums of
(logZ - gold score) are returned and summed/averaged on the host.

Normalizer (the sequential part): forward and backward CRF recurrences in
probability space meet at the sequence midpoint, halving the chain to 511
steps. Both chains are stacked in one [112, 64] tile (fwd states at
partitions 0-47, bwd at 64-111 so the final bwd-only matmul lands on a legal
PE tile boundary) and advance together: one bf16 matmul against a constant
block-diagonal [112,112] stationary + one DVE multiply with bulk-
pre-exponentiated emissions, exp(em - 4.9375) (constant centers the growth).
Range control: every 64 steps a per-column sum is taken with a ones-vector
matmul, inverted on DVE, broadcast with a K=1 matmul, and folded into the
emission slab 4 steps later - fully off the critical chain (the scale
commutes through the matmul). Log corrections accumulate via a deferred
Ln pass at the end.

Gold-path score: tag one-hots are built with packed bf16 is_equal ops into
an interleaved [em_j | oh_j] per-step layout so that the moving window
[oh_{j-1} | em_j] is a single contiguous 96-column slice.  One PE matmul per
timestep (stationary oh_j) then accumulates BOTH the transition-pair count
matrix (cols 0-47) and the gathered-emission matrix (cols 48-95) into one
[48,96] PSUM bank across all 512 steps; a tiny elementwise pass against
[transitions | I] finishes the score.  The matmuls are paced one per scan
step in program order so they hide inside the PE's idle window without
stretching the chain.  Start/end lookups are tiny one-hot matmul dots.

All partition-axis reductions are ones-vector matmuls (gpsimd C-reduce is
~5-13us per op on this path and partition_broadcast / indirect_copy /
tensor_tensor_reduce fail walrus codegen entirely).
"""
import os
import sys

import numpy as np
import ml_dtypes

for _p in ("/opt/trn_rl_repo", "/root/.axon_site/_ro/trn_rl_repo"):
    if os.path.isdir(_p) and _p not in sys.path:
        sys.path.insert(0, _p)

import concourse.bass as bass
import concourse.bacc as bacc
import concourse.mybir as mybir
import concourse.tile as tile

# Enable walrus's redundant-weight-load elision: the scan reuses one
# stationary for 511 matmuls and the default (disabled) reloads it each time.
if os.environ.get("LDW_OPT", "0") == "1":
    from concourse import bass_utils as _bu

    _orig_run_command = _bu.run_command

    def _run_command_ldw(argv, **kw):
        argv = ["--enable-ldw-opt=true" if a == "--enable-ldw-opt=false" else a
                for a in argv]
        return _orig_run_command(argv, **kw)

    _bu.run_command = _run_command_ldw

B, S, T = 512, 1024, 48
NCORES = 8
BL = B // NCORES  # 64
NSTEPS = (S - 2) // 2  # 511 paired fwd/bwd loop iterations
CBIAS = 4.9375  # constant folded into exp() of each step's emissions
RK = 64  # renorm cadence
NREN = (NSTEPS - 1) // RK  # 7 renorms at k = 64, 128, ..., 448
CHUNK = 64  # scan slabs per DMA/exp chunk (8 chunks of 64 slabs)
NC = 8  # numerator chunks (64 steps each)
NJ = 64  # steps per numerator chunk
NB = 96  # per-step block stride in the interleaved numerator layout
CW = 48 + NJ * NB  # comb tile width: leading oh_{-1} pad + 64 blocks
SL = 4  # steps per one-hot is_equal slice

BF16 = mybir.dt.bfloat16
F32 = mybir.dt.float32
AL = mybir.AluOpType
AX = mybir.AxisListType
AF = mybir.ActivationFunctionType

bf16np = ml_dtypes.bfloat16


def _build_graph():
    nc = bacc.Bacc("TRN2", target_bir_lowering=False, debug=False)

    emstack = nc.dram_tensor("emstack", [112, 512 * BL], BF16, kind="ExternalInput")
    slab0 = nc.dram_tensor("slab0", [96, BL], F32, kind="ExternalInput")
    bias96 = nc.dram_tensor("bias96", [96, 1], F32, kind="ExternalInput")
    transT = nc.dram_tensor("transT", [T, T], F32, kind="ExternalInput")
    transN = nc.dram_tensor("transN", [T, T], F32, kind="ExternalInput")
    emint = nc.dram_tensor("emint", [128, NC * CW], BF16, kind="ExternalInput")
    tagsnat = nc.dram_tensor("tagsnat", [BL, S], BF16, kind="ExternalInput")
    tpcrep = nc.dram_tensor("tpcrep", [128, 512 * T], BF16, kind="ExternalInput")
    tprev0 = nc.dram_tensor("tprev0", [128, NC], BF16, kind="ExternalInput")
    iotabd = nc.dram_tensor("iotabd", [128, T], BF16, kind="ExternalInput")
    iorepd = nc.dram_tensor("iorepd", [128, SL * T], BF16, kind="ExternalInput")
    numcoefd = nc.dram_tensor("numcoefd", [T, 2 * T], F32, kind="ExternalInput")
    startv = nc.dram_tensor("startv", [T, 1], F32, kind="ExternalInput")
    endv = nc.dram_tensor("endv", [T, 1], F32, kind="ExternalInput")
    outd = nc.dram_tensor("out", [1, 1], F32, kind="ExternalOutput")

    with tile.TileContext(nc) as tc:
        _kern(tc, nc, emstack, slab0, bias96, transT, transN, emint, tagsnat,
              tpcrep, tprev0, iotabd, iorepd, numcoefd, startv, endv, outd)
    nc.compile()
    return nc


def _kern(tc, nc, emstack, slab0, bias96, transT, transN, emint, tagsnat,
          tpcrep, tprev0, iotabd, iorepd, numcoefd, startv, endv, outd):
    from contextlib import ExitStack
    ctx = ExitStack()
    const = ctx.enter_context(tc.tile_pool(name="const", bufs=1))
    statep = ctx.enter_context(tc.tile_pool(name="state", bufs=4))
    psp = ctx.enter_context(tc.tile_pool(name="psp", bufs=3, space="PSUM"))
    psx = ctx.enter_context(tc.tile_pool(name="psx", bufs=1, space="PSUM"))
    psn = ctx.enter_context(tc.tile_pool(name="psn", bufs=1, space="PSUM"))
    rawp = ctx.enter_context(tc.tile_pool(name="raw", bufs=3))
    expdp = ctx.enter_context(tc.tile_pool(name="expd", bufs=2))
    combp = ctx.enter_context(tc.tile_pool(name="comb", bufs=2))
    trcp = ctx.enter_context(tc.tile_pool(name="trc", bufs=2))
    smallp = ctx.enter_context(tc.tile_pool(name="small", bufs=1))

    # ---------- constants / small inputs ----------
    bigm = const.tile([112, 112], BF16)
    trT = const.tile([T, T], F32)
    trS = const.tile([112, T], F32)
    biasT = const.tile([112, 1], F32)
    stS = const.tile([T, 1], F32)
    enS = const.tile([T, 1], F32)
    sl0 = const.tile([112, BL], F32)
    tagS = const.tile([BL, S], BF16)
    iotab = const.tile([128, T], BF16)
    iorep = const.tile([128, SL * T], BF16)
    tprev = const.tile([128, NC], BF16)
    numcoef = const.tile([T, 2 * T], F32)
    onescol = const.tile([112, 1], BF16)

    ones48 = const.tile([T, 1], BF16)
    ones64 = const.tile([BL, 1], BF16)
    onesrow = const.tile([1, 112], F32)
    nc.vector.memset(onesrow[:], 1.0)
    mstore = const.tile([1, RK * 8], F32)
    cbias = const.tile([112, 1], F32)
    nc.vector.memset(cbias[:], -CBIAS)
    finc = const.tile([1, 1], F32)
    nc.vector.memset(finc[:], float(BL * 2 * NSTEPS * CBIAS))

    # DMA emission order == single-FIFO-queue service order: scan-critical
    # transfers first, numerator streams strictly after the early scan chunks.
    raw0 = rawp.tile([112, CHUNK * BL], BF16, tag="raw")
    nc.sync.dma_start(raw0[0:112, 0:8 * BL], emstack[:, 0:8 * BL])
    nc.vector.memset(biasT[:], 0.0)
    nc.vector.memset(sl0[:], 0.0)
    nc.sync.dma_start(biasT[0:T, :], bias96[0:T, :])
    nc.sync.dma_start(biasT[64:112, :], bias96[T:96, :])
    nc.sync.dma_start(sl0[0:T, :], slab0[0:T, :])
    nc.sync.dma_start(sl0[64:112, :], slab0[T:96, :])
    nc.sync.dma_start(trT[:], transT[:, :])
    nc.sync.dma_start(trS[64:112, :], transN[:, :])
    # warm the ACT Exp table while the first transfers are in flight; the Ln
    # table load is stamped into the scan body so it never gates startup
    actwarm = const.tile([1, 1], F32)
    nc.vector.memset(actwarm[:], 1.0)
    nc.scalar.activation(actwarm[:], actwarm[:], AF.Exp)
    with tc.tile_wait_until(12):
        nc.scalar.activation(actwarm[:], actwarm[:], AF.Ln)
    # scan chunks 1-2 ahead of any numerator bytes (rawp bufs=3 keeps the
    # FIFO head unblocked)
    raw_tiles = {0: raw0}
    for ci, (c0, clen) in ((1, (8, 16)), (2, (24, 32))):
        rw = rawp.tile([112, CHUNK * BL], BF16, tag="raw")
        nc.sync.dma_start(rw[0:112, 0:clen * BL],
                          emstack[:, c0 * BL:(c0 + clen) * BL])
        raw_tiles[ci] = rw


    nc.vector.memset(onescol[:], 1.0)
    nc.vector.memset(ones48[:], 1.0)
    nc.vector.memset(ones64[:], 1.0)
    nc.vector.memset(mstore[:], 1.0)  # unused slots log to 0

    # blockdiag([exp(transT), exp(transN)]) in bf16; stamps pin the ACT order
    # (state init and bigm ahead of the chunk exps) so the first scan matmul's
    # cumulative ACT threshold stays minimal
    nc.vector.memset(bigm[:], 0.0)
    with tc.tile_wait_until(0.02):
        nc.scalar.activation(bigm[0:T, 0:T], trT[:], AF.Exp)
        nc.scalar.activation(bigm[64:112, 64:112], trS[64:112, :], AF.Exp)

    # ---------- state init: exp(slab0 + [start;end]) ----------
    state = statep.tile([112, BL], BF16, tag="state")
    with tc.tile_wait_until(0.01):
        nc.scalar.activation(state[:], sl0[:], AF.Exp, bias=biasT[:])

    # numerator chunk-0 streams (after the scan-critical DMAs)
    comb_tiles = {}
    trc_tiles = {}

    def num_chunk_dma(c, split=False):
        cb = combp.tile([128, CW], BF16, tag="comb")
        tr = trcp.tile([128, NJ * T], BF16, tag="trc")
        if split:
            # halved transfers so the first slices unblock ~3us earlier
            h = 48 + (NJ // 2) * NB
            nc.gpsimd.dma_start(cb[:, 0:h], emint[:, c * CW:c * CW + h])
            nc.gpsimd.dma_start(tr[:, 0:NJ * T // 2],
                                tpcrep[:, c * NJ * T:c * NJ * T + NJ * T // 2])
            # slice-constants land between the halves (needed by slice 0)
            nc.gpsimd.dma_start(iotab[:], iotabd[:, :])
            nc.gpsimd.dma_start(iorep[:], iorepd[:, :])
            nc.gpsimd.dma_start(tprev[:], tprev0[:, :])
            nc.gpsimd.dma_start(cb[:, h:CW], emint[:, c * CW + h:(c + 1) * CW])
            nc.gpsimd.dma_start(tr[:, NJ * T // 2:NJ * T],
                                tpcrep[:, c * NJ * T + NJ * T // 2:(c + 1) * NJ * T])
        else:
            nc.gpsimd.dma_start(cb[:], emint[:, c * CW:(c + 1) * CW])
            nc.gpsimd.dma_start(tr[:], tpcrep[:, c * NJ * T:(c + 1) * NJ * T])
        comb_tiles[c] = cb
        trc_tiles[c] = tr

    num_chunk_dma(0, split=True)
    nc.gpsimd.dma_start(stS[:], startv[:, :])
    nc.gpsimd.dma_start(enS[:], endv[:, :])
    nc.gpsimd.dma_start(numcoef[:], numcoefd[:, :])
    nc.gpsimd.dma_start(tagS[:], tagsnat[:, :])

    psBig = psn.tile([T, 2 * T], F32, tag="psBig")

    def num_slice(s):
        """Build one-hots for steps [SL*s, SL*(s+1)); boundary col at s%16==0.

        Ordered BEFORE a scan mul in DVE program order so it executes inside
        the mul's semaphore-wait window and never extends the chain.
        """
        c = (SL * s) // NJ
        cb = comb_tiles[c]
        if (SL * s) % NJ == 0:
            nc.vector.tensor_tensor(
                cb[:, 0:T], iotab[:], tprev[:, c:c + 1].broadcast_to([128, T]),
                op=AL.is_equal)
        s0 = (SL * s) % NJ
        oh_view = cb[:, 48:].rearrange("p (j w) -> p j w", w=NB)
        nc.vector.tensor_tensor(
            oh_view[:, s0:s0 + SL, T:NB],
            iorep[:, 0:SL * T].rearrange("p (j w) -> p j w", w=T),
            trc_tiles[c][:, s0 * T:(s0 + SL) * T].rearrange(
                "p (j w) -> p j w", w=T),
            op=AL.is_equal)

    def num_mm(j):
        """Single count/emission matmul for step j (PE, after the scan mm)."""
        c, jj = j // NJ, j % NJ
        cb = comb_tiles[c]
        nc.tensor.matmul(psBig[:], cb[:, NB * jj + 96:NB * jj + 144],
                         cb[:, NB * jj:NB * jj + NB],
                         start=(j == 0), stop=(j == 511), skip_group_check=True)

    # ---------- main scan loop ----------
    # chunk boundaries: small leading chunks so the scan starts early
    bounds = [0, 8, 24, 56] + [56 + 64 * i for i in range(1, 8)] + [512]
    start_of = {}
    for ci in range(len(bounds) - 1):
        for p in range(bounds[ci], bounds[ci + 1]):
            start_of[p] = (ci, bounds[ci], bounds[ci + 1] - bounds[ci])
    expd_tiles = {}
    pend = None  # deferred renorm: (apply_at_k, bcastP)
    nren = 0
    CONS_LAG = 16  # scan steps between one-hot availability and its matmul
    PROD_LEAD = 8  # scan steps a one-hot slice is produced ahead of use
    mm_pending = []
    next_slice = 0
    next_mm = 0
    for k in range(1, NSTEPS + 1):
        ci, c0, clen = start_of[k - 1]
        if k - 1 == c0:
            if ci in raw_tiles:
                raw = raw_tiles[ci]
            else:
                raw = rawp.tile([112, CHUNK * BL], BF16, tag="raw")
                nc.sync.dma_start(raw[0:112, 0:clen * BL],
                                  emstack[:, c0 * BL:(c0 + clen) * BL])
            expd = expdp.tile([112, CHUNK * BL], BF16, tag="expd")
            with tc.tile_wait_until(0.1 + 0.01 * ci):
                nc.scalar.activation(expd[0:112, 0:clen * BL],
                                     raw[0:112, 0:clen * BL], AF.Exp,
                                     bias=cbias[:])
            expd_tiles[ci] = expd
        expd = expd_tiles[ci]
        j = (k - 1) - c0

        # numerator chunk DMA: emit when the previous chunk starts consuming
        # (unstamped: the scheduler hoists it as early as pool buffers allow)
        if (k - CONS_LAG - 1) % NJ == 0 and 1 <= (k - CONS_LAG - 1) // NJ + 1 < NC:
            num_chunk_dma((k - CONS_LAG - 1) // NJ + 1)

        # logical-time stamps pin the per-step interleave: the Tile scheduler
        # otherwise bunches all numerator matmuls into the earliest steps,
        # stretching the latency chain (measured +35us).
        with tc.tile_wait_until(k):
            ps = psp.tile([112, BL], F32, tag="ps")
            nc.tensor.matmul(ps[:], bigm[:], state[:], start=True, stop=True)

        # numerator matmuls ride the PE queue right behind the scan matmul
        # (renorm steps and the bcast-matmul slot two steps later stay clear)
        if k % RK != 0 and (k - 2) % RK != 0:
            budget = 2
            while mm_pending and budget:
                with tc.tile_wait_until(k + 0.25):
                    num_mm(mm_pending.pop(0))
                budget -= 1
        if k - 1 - CONS_LAG >= 0 and next_mm <= k - 1 - CONS_LAG:
            mm_pending.append(next_mm)
            next_mm += 1

        # one-hot production on DVE, ordered BEFORE this step's scan mul
        if k - 1 - CONS_LAG + PROD_LEAD >= SL * next_slice and next_slice < 512 // SL:
            with tc.tile_wait_until(k + 0.4):
                num_slice(next_slice)
            next_slice += 1

        nstate = statep.tile([112, BL], BF16, tag="state")
        eop = expd[:, j * BL:(j + 1) * BL]
        if pend is not None and pend[0] == k:
            with tc.tile_wait_until(k + 0.45):
                esc = expdp.tile([112, BL], BF16, tag="esc")
                nc.vector.tensor_mul(esc[:], eop, pend[1][:])
            eop = esc[:]
            pend = None
        with tc.tile_wait_until(k + 0.5):
            nc.vector.tensor_mul(nstate[:], ps[:], eop)
        state = nstate

        if k % RK == 0 and k < NSTEPS:
            # off-chain: col-sum -> recip -> broadcast; applied at step k+4.
            # Spread over steps k..k+2 so no single DVE/PE window overflows:
            # col-sum mm at k, mx copy on ACT, fast reciprocal in step k+1's
            # DVE window, K=1 broadcast mm in step k+2's numerator slot.
            nren += 1
            with tc.tile_wait_until(k + 0.6):
                mxp = psx.tile([1, BL], F32, tag="mxp")
                nc.tensor.matmul(mxp[:], onescol[:], state[:], start=True, stop=True)
            with tc.tile_wait_until(k + 0.7):
                mx = mstore[0:1, (nren - 1) * BL:nren * BL]
                nc.scalar.activation(mx, mxp[:], AF.Copy)
            with tc.tile_wait_until(k + 1.6):
                rcp = smallp.tile([1, BL], F32, tag="rcp")
                nc.vector.reciprocal_approx_fast(rcp[:], mxp[:])
            with tc.tile_wait_until(k + 2.25):
                bcast = psx.tile([112, BL], F32, tag="bcp")
                nc.tensor.matmul(bcast[:], onesrow[:], rcp[:], start=True, stop=True)
            pend = (k + 4, bcast)

    # leftover numerator work (CONS_LAG tail) overlaps the final combine
    with tc.tile_wait_until(NSTEPS + 1):
        while next_slice < 512 // SL:
            num_slice(next_slice)
            next_slice += 1
        while next_mm <= 511:
            mm_pending.append(next_mm)
            next_mm += 1
        for j2 in mm_pending:
            num_mm(j2)

    # renorm-log pass: mstore is complete after the last renorm (k=448);
    # stamp it into the scan body so the tail doesn't pay the Ln + reduce
    with tc.tile_wait_until(RK * NREN + 8.5):
        lnm = smallp.tile([1, RK * 8], F32, tag="lnm")
        nc.scalar.activation(lnm[:], mstore[:], AF.Ln)
        carry = smallp.tile([1, BL], F32, tag="carry")
        nc.vector.tensor_reduce(
            carry[:], lnm[0:1, :].rearrange("p (j b) -> p b j", j=8), axis=AX.X,
            op=AL.add)

    # ---------- numerator: start/end lookups ----------
    ctx.enter_context(tc.tile_wait_until(NSTEPS + 2))

    def edge_dot(tag_col, vec, name):
        oh0 = smallp.tile([BL, T], BF16, tag=f"oh0{name}")
        i2 = iotab[0:BL, :]
        t2 = tag_col.broadcast_to([BL, T])
        nc.vector.tensor_tensor(oh0[:], i2, t2, op=AL.is_equal)
        cnt = psx.tile([T, 1], F32, tag="cnt")
        nc.tensor.matmul(cnt[:], oh0[:], ones64[:], start=True, stop=True)
        dots = smallp.tile([T, 1], BF16, tag=f"dots{name}")
        with nc.allow_low_precision(reason="scalar total; tolerant"):
            nc.vector.tensor_mul(dots[:], cnt[:], vec[:])
        ssump = psx.tile([1, 1], F32, tag="sum")
        nc.tensor.matmul(ssump[:], dots[:], ones48[:], start=True, stop=True)
        ssum = smallp.tile([1, 1], F32, tag=f"ssum{name}")
        nc.scalar.activation(ssum[:], ssump[:], AF.Copy)
        return ssum

    stsum = edge_dot(tagS[:, 0:1], stS, "st")
    ensum = edge_dot(tagS[:, S - 1:S], enS, "en")

    # ---------- combine fwd/bwd: Z = a_511 . (M @ g_512) ----------
    psf = psx.tile([T, BL], F32, tag="bcp")
    nc.tensor.matmul(psf[:], bigm[64:112, 64:112], state[64:112, :], start=True,
                     stop=True)
    stateF = smallp.tile([T, BL], F32, tag="stateF")
    nc.scalar.activation(stateF[:], state[0:T, :], AF.Copy)
    z1 = smallp.tile([T, BL], BF16, tag="z1")
    with nc.allow_low_precision(reason="z products; log tolerant"):
        nc.vector.tensor_mul(z1[:], stateF[:], psf[:])
    pz = psx.tile([1, BL], F32, tag="mxp")
    nc.tensor.matmul(pz[:], ones48[:], z1[:], start=True, stop=True)

    lz = smallp.tile([1, BL], F32, tag="lz")
    nc.scalar.activation(lz[:], pz[:], AF.Ln)
    # logZ = lz + 2*carry + 2*NSTEPS*CBIAS
    lzc = smallp.tile([1, BL], F32, tag="lzc")
    nc.vector.scalar_tensor_tensor(lzc[:], carry[:], 2.0, lz[:], op0=AL.mult, op1=AL.add)
    lzsum = smallp.tile([1, 1], F32, tag="lzsum")
    nc.vector.tensor_reduce(lzsum[:], lzc[:], axis=AX.X, op=AL.add)

    # ---------- numerator finalize: sum(psBig * [transitions | I]) ----------
    ct = smallp.tile([T, 2 * T], F32, tag="ct")
    nc.vector.tensor_mul(ct[:], psBig[:], numcoef[:])
    ctr = smallp.tile([T, 1], F32, tag="ctr")
    nc.vector.tensor_reduce(ctr[:], ct[:], axis=AX.X, op=AL.add)
    ctrb = smallp.tile([T, 1], BF16, tag="ctrb")
    with nc.allow_low_precision(reason="scalar total; tolerant"):
        nc.vector.tensor_copy(ctrb[:], ctr[:])
    ctsump = psx.tile([1, 1], F32, tag="sum")
    nc.tensor.matmul(ctsump[:], ctrb[:], ones48[:], start=True, stop=True)
    ctsum = smallp.tile([1, 1], F32, tag="ctsum")
    nc.scalar.activation(ctsum[:], ctsump[:], AF.Copy)

    # ---------- total = lzsum + finc - ctsum - stsum - ensum ----------
    t1 = smallp.tile([1, 1], F32, tag="t1")
    nc.vector.scalar_tensor_tensor(t1[:], lzsum[:], finc[:], ctsum[:],
                                   op0=AL.add, op1=AL.subtract)
    t2 = smallp.tile([1, 1], F32, tag="t2")
    nc.vector.scalar_tensor_tensor(t2[:], t1[:], stsum[:], ensum[:],
                                   op0=AL.subtract, op1=AL.subtract)
    nc.sync.dma_start(outd[:, :], t2[:])
    ctx.close()


def _prep_core_inputs(em, tags, transitions, start, end):
    """em [BL,S,T] f32, tags [BL,S] int64 -> dict of device arrays."""
    em = np.asarray(em, dtype=np.float32)
    tags = np.asarray(tags).astype(np.int32)

    # emstack [112, 512*BL]: pos j holds slab (j+1): upper em[:,j+1,:]^T,
    # lower em[:,1022-j,:]^T; pos 511 is padding.
    emstack = np.zeros((112, 512, BL), dtype=np.float32)
    emstack[0:T, 0:NSTEPS] = em[:, 1:NSTEPS + 1, :].transpose(2, 1, 0)
    emstack[64:112, 0:NSTEPS] = em[:, S - 2:S - 2 - NSTEPS:-1, :].transpose(2, 1, 0)
    emstack = emstack.reshape(112, 512 * BL).astype(bf16np)

    slab0 = np.concatenate([em[:, 0, :].T, em[:, S - 1, :].T], axis=0).astype(np.float32)
    bias96 = np.concatenate([start, end])[:, None].astype(np.float32)

    # interleaved numerator emissions: per chunk c, 48-col pad then 64 blocks
    # of [em_j (48) | oh_j slot (48, zero)]
    emint = np.zeros((BL * 2, NC, CW), dtype=bf16np)
    emr = em.reshape(BL, 2, NC, NJ, T).reshape(BL * 2, NC, NJ, T)
    emint[:, :, 48:].reshape(BL * 2, NC, NJ, NB)[:, :, :, 0:T] = emr
    emint = emint.reshape(128, NC * CW)

    tpcur = tags.reshape(BL, 2, 512).reshape(BL * 2, 512)
    # prev-tag value for step j=0 of each chunk (sentinel T for (h=0, c=0))
    tprev0 = np.zeros((BL * 2, NC), dtype=np.int32)
    for c in range(NC):
        if c == 0:
            tprev0[0::2, 0] = T
            tprev0[1::2, 0] = tags[:, 511]
        else:
            tprev0[:, c] = tpcur[:, c * NJ - 1]

    iotab = np.tile(np.arange(T, dtype=np.float32), (128, 1))
    iorep = np.tile(np.arange(T, dtype=np.float32), (128, SL))
    numcoef = np.concatenate(
        [np.asarray(transitions, dtype=np.float32), np.eye(T, dtype=np.float32)],
        axis=1)

    return {
        "emstack": emstack,
        "slab0": slab0,
        "bias96": bias96,
        "transT": np.ascontiguousarray(transitions.T).astype(np.float32),
        "transN": np.ascontiguousarray(transitions).astype(np.float32),
        "emint": emint,
        "tagsnat": tags.astype(bf16np),
        "tpcrep": np.repeat(tpcur.astype(bf16np), T).reshape(128, 512 * T),
        "tprev0": tprev0.astype(bf16np),
        "iotabd": iotab.astype(bf16np),
        "iorepd": iorep.astype(bf16np),
        "numcoefd": numcoef,
        "startv": start[:, None].astype(np.float32),
        "endv": end[:, None].astype(np.float32),
    }


def prep_all_inputs(emissions, tags, mask, transitions, start_transitions,
                    end_transitions):
    em = np.asarray(emissions, dtype=np.float32)
    tg = np.asarray(tags)
    tr = np.asarray(transitions, dtype=np.float32)
    st = np.asarray(start_transitions, dtype=np.float32)
    en = np.asarray(end_transitions, dtype=np.float32)
    return [
        _prep_core_inputs(em[c * BL:(c + 1) * BL], tg[c * BL:(c + 1) * BL], tr, st, en)
        for c in range(NCORES)
    ]


_NC_CACHE = {}


def get_graph():
    if "nc" not in _NC_CACHE:
        _NC_CACHE["nc"] = _build_graph()
    return _NC_CACHE["nc"]


def kernel(emissions, tags, mask, transitions, start_transitions, end_transitions,
           **kw):
    from concourse import bass_utils
    nc = get_graph()
    in_maps = prep_all_inputs(emissions, tags, mask, transitions,
                              start_transitions, end_transitions)
    res = bass_utils.run_bass_kernel_spmd(nc, in_maps, core_ids=list(range(NCORES)))
    total = sum(float(res.results[c]["out"][0, 0]) for c in range(NCORES))
    return np.float32(total / B)


if __name__ == "__main__":
    get_graph()
    print("graph built ok")


# revision 25
# speedup vs baseline: 1.0482x; 1.0143x over previous
"""Trainium2 Bass kernel for CRF negative log-likelihood loss (nn_CRF).

Sharding: data-parallel, 8 cores x 64 batch rows; per-core partial sums of
(logZ - gold score) are returned and summed/averaged on the host.

Normalizer (the sequential part): forward and backward CRF recurrences in
probability space meet at the sequence midpoint, halving the chain to 511
steps. Both chains are stacked in one [112, 64] tile (fwd states at
partitions 0-47, bwd at 64-111 so the final bwd-only matmul lands on a legal
PE tile boundary) and advance together: one bf16 matmul against a constant
block-diagonal [112,112] stationary + one DVE multiply with bulk-
pre-exponentiated emissions, exp(em - 4.9375) (constant centers the growth).
Range control: every 64 steps a per-column sum is taken with a ones-vector
matmul, inverted on DVE, broadcast with a K=1 matmul, and folded into the
emission slab 4 steps later - fully off the critical chain (the scale
commutes through the matmul). Log corrections accumulate via a deferred
Ln pass at the end.

Gold-path score: tag one-hots are built with packed bf16 is_equal ops into
an interleaved [em_j | oh_j] per-step layout so that the moving window
[oh_{j-1} | em_j] is a single contiguous 96-column slice.  One PE matmul per
timestep (stationary oh_j) then accumulates BOTH the transition-pair count
matrix (cols 0-47) and the gathered-emission matrix (cols 48-95) into one
[48,96] PSUM bank across all 512 steps; a tiny elementwise pass against
[transitions | I] finishes the score.  The matmuls are paced one per scan
step in program order so they hide inside the PE's idle window without
stretching the chain.  Start/end lookups are tiny one-hot matmul dots.

All partition-axis reductions are ones-vector matmuls (gpsimd C-reduce is
~5-13us per op on this path and partition_broadcast / indirect_copy /
tensor_tensor_reduce fail walrus codegen entirely).
"""
import os
import sys

import numpy as np
import ml_dtypes

for _p in ("/opt/trn_rl_repo", "/root/.axon_site/_ro/trn_rl_repo"):
    if os.path.isdir(_p) and _p not in sys.path:
        sys.path.insert(0, _p)

import concourse.bass as bass
import concourse.bacc as bacc
import concourse.mybir as mybir
import concourse.tile as tile

# Enable walrus's redundant-weight-load elision: the scan reuses one
# stationary for 511 matmuls and the default (disabled) reloads it each time.
if os.environ.get("LDW_OPT", "0") == "1":
    from concourse import bass_utils as _bu

    _orig_run_command = _bu.run_command

    def _run_command_ldw(argv, **kw):
        argv = ["--enable-ldw-opt=true" if a == "--enable-ldw-opt=false" else a
                for a in argv]
        return _orig_run_command(argv, **kw)

    _bu.run_command = _run_command_ldw

B, S, T = 512, 1024, 48
NCORES = 8
BL = B // NCORES  # 64
NSTEPS = (S - 2) // 2  # 511 paired fwd/bwd loop iterations
CBIAS = 4.9375  # constant folded into exp() of each step's emissions
RK = 64  # renorm cadence
NREN = (NSTEPS - 1) // RK  # 7 renorms at k = 64, 128, ..., 448
CHUNK = 64  # scan slabs per DMA/exp chunk (8 chunks of 64 slabs)
NC = 8  # numerator chunks (64 steps each)
NJ = 64  # steps per numerator chunk
NB = 96  # per-step block stride in the interleaved numerator layout
CW = 48 + NJ * NB  # comb tile width: leading oh_{-1} pad + 64 blocks
SL = 4  # steps per one-hot is_equal slice
SPW = 115 + 2 * T  # packed small-constant tensor width

BF16 = mybir.dt.bfloat16
F32 = mybir.dt.float32
AL = mybir.AluOpType
AX = mybir.AxisListType
AF = mybir.ActivationFunctionType

bf16np = ml_dtypes.bfloat16


def _build_graph():
    nc = bacc.Bacc("TRN2", target_bir_lowering=False, debug=False)

    emstack = nc.dram_tensor("emstack", [112, 512 * BL], BF16, kind="ExternalInput")
    smallpkd = nc.dram_tensor("smallpkd", [112, SPW], F32, kind="ExternalInput")
    emint = nc.dram_tensor("emint", [128, NC * CW], BF16, kind="ExternalInput")
    tagsnat = nc.dram_tensor("tagsnat", [BL, S], BF16, kind="ExternalInput")
    tpcrep = nc.dram_tensor("tpcrep", [128, 512 * T], BF16, kind="ExternalInput")
    tprev0 = nc.dram_tensor("tprev0", [128, NC], BF16, kind="ExternalInput")
    iotabd = nc.dram_tensor("iotabd", [128, T], BF16, kind="ExternalInput")
    iorepd = nc.dram_tensor("iorepd", [128, SL * T], BF16, kind="ExternalInput")
    outd = nc.dram_tensor("out", [1, 1], F32, kind="ExternalOutput")

    with tile.TileContext(nc) as tc:
        _kern(tc, nc, emstack, smallpkd, emint, tagsnat,
              tpcrep, tprev0, iotabd, iorepd, outd)
    nc.compile()
    return nc


def _kern(tc, nc, emstack, smallpkd, emint, tagsnat,
          tpcrep, tprev0, iotabd, iorepd, outd):
    from contextlib import ExitStack
    ctx = ExitStack()
    const = ctx.enter_context(tc.tile_pool(name="const", bufs=1))
    statep = ctx.enter_context(tc.tile_pool(name="state", bufs=4))
    psp = ctx.enter_context(tc.tile_pool(name="psp", bufs=3, space="PSUM"))
    psx = ctx.enter_context(tc.tile_pool(name="psx", bufs=1, space="PSUM"))
    psn = ctx.enter_context(tc.tile_pool(name="psn", bufs=1, space="PSUM"))
    rawp = ctx.enter_context(tc.tile_pool(name="raw", bufs=3))
    expdp = ctx.enter_context(tc.tile_pool(name="expd", bufs=2))
    combp = ctx.enter_context(tc.tile_pool(name="comb", bufs=2))
    trcp = ctx.enter_context(tc.tile_pool(name="trc", bufs=2))
    smallp = ctx.enter_context(tc.tile_pool(name="small", bufs=1))

    # ---------- constants / small inputs ----------
    bigm = const.tile([112, 112], BF16)
    smallpk = const.tile([112, SPW], F32)
    biasT = smallpk[:, 0:1]
    sl0 = smallpk[:, 1:65]
    trpk = smallpk[:, 65:113]  # rows 0-47: transitions.T; rows 64-111: transitions
    stS = smallpk[0:T, 113:114]
    enS = smallpk[0:T, 114:115]
    numcoef = smallpk[0:T, 115:115 + 2 * T]
    tagS = const.tile([BL, S], BF16)
    iotab = const.tile([128, T], BF16)
    iorep = const.tile([128, SL * T], BF16)
    tprev = const.tile([128, NC], BF16)
    onescol = const.tile([112, 1], BF16)

    ones48 = const.tile([T, 1], BF16)
    ones64 = const.tile([BL, 1], BF16)
    onesrow = const.tile([1, 112], F32)
    nc.vector.memset(onesrow[:], 1.0)
    mstore = const.tile([1, RK * 8], F32)
    cbias = const.tile([112, 1], F32)
    nc.vector.memset(cbias[:], -CBIAS)
    finc = const.tile([1, 1], F32)
    nc.vector.memset(finc[:], float(BL * 2 * NSTEPS * CBIAS))

    # DMA emission order == single-FIFO-queue service order: scan-critical
    # transfers first, numerator streams strictly after the early scan chunks.
    raw0 = rawp.tile([112, CHUNK * BL], BF16, tag="raw")
    nc.sync.dma_start(smallpk[:], smallpkd[:, :])
    nc.sync.dma_start(raw0[0:112, 0:8 * BL], emstack[:, 0:8 * BL])
    # warm the ACT Exp table while the first transfers are in flight; the Ln
    # table load is stamped into the scan body so it never gates startup
    actwarm = const.tile([1, 1], F32)
    nc.vector.memset(actwarm[:], 1.0)
    nc.scalar.activation(actwarm[:], actwarm[:], AF.Exp)
    with tc.tile_wait_until(12):
        nc.scalar.activation(actwarm[:], actwarm[:], AF.Ln)
    # scan chunks 1-2 ahead of any numerator bytes (rawp bufs=3 keeps the
    # FIFO head unblocked)
    raw_tiles = {0: raw0}
    for ci, (c0, clen) in ((1, (8, 16)), (2, (24, 32))):
        rw = rawp.tile([112, CHUNK * BL], BF16, tag="raw")
        nc.sync.dma_start(rw[0:112, 0:clen * BL],
                          emstack[:, c0 * BL:(c0 + clen) * BL])
        raw_tiles[ci] = rw


    nc.vector.memset(onescol[:], 1.0)
    nc.vector.memset(ones48[:], 1.0)
    nc.vector.memset(ones64[:], 1.0)
    nc.vector.memset(mstore[:], 1.0)  # unused slots log to 0

    # blockdiag([exp(transT), exp(transN)]) in bf16; stamps pin the ACT order
    # (state init and bigm ahead of the chunk exps) so the first scan matmul's
    # cumulative ACT threshold stays minimal
    nc.vector.memset(bigm[:], 0.0)
    with tc.tile_wait_until(0.02):
        nc.scalar.activation(bigm[0:T, 0:T], trpk[0:T, :], AF.Exp)
        nc.scalar.activation(bigm[64:112, 64:112], trpk[64:112, :], AF.Exp)

    # ---------- state init: exp(slab0 + [start;end]) ----------
    state = statep.tile([112, BL], BF16, tag="state")
    with tc.tile_wait_until(0.01):
        nc.scalar.activation(state[:], sl0, AF.Exp, bias=biasT)

    # numerator chunk-0 streams (after the scan-critical DMAs)
    comb_tiles = {}
    trc_tiles = {}

    def num_chunk_dma(c, split=False):
        cb = combp.tile([128, CW], BF16, tag="comb")
        tr = trcp.tile([128, NJ * T], BF16, tag="trc")
        if split:
            # halved transfers so the first slices unblock ~3us earlier
            h = 48 + (NJ // 2) * NB
            nc.gpsimd.dma_start(cb[:, 0:h], emint[:, c * CW:c * CW + h])
            nc.gpsimd.dma_start(tr[:, 0:NJ * T // 2],
                                tpcrep[:, c * NJ * T:c * NJ * T + NJ * T // 2])
            # slice-constants land between the halves (needed by slice 0)
            nc.gpsimd.dma_start(iotab[:], iotabd[:, :])
            nc.gpsimd.dma_start(iorep[:], iorepd[:, :])
            nc.gpsimd.dma_start(tprev[:], tprev0[:, :])
            nc.gpsimd.dma_start(cb[:, h:CW], emint[:, c * CW + h:(c + 1) * CW])
            nc.gpsimd.dma_start(tr[:, NJ * T // 2:NJ * T],
                                tpcrep[:, c * NJ * T + NJ * T // 2:(c + 1) * NJ * T])
        else:
            nc.gpsimd.dma_start(cb[:], emint[:, c * CW:(c + 1) * CW])
            nc.gpsimd.dma_start(tr[:], tpcrep[:, c * NJ * T:(c + 1) * NJ * T])
        comb_tiles[c] = cb
        trc_tiles[c] = tr

    num_chunk_dma(0, split=True)
    nc.gpsimd.dma_start(tagS[:], tagsnat[:, :])

    psBig = psn.tile([T, 2 * T], F32, tag="psBig")

    def num_slice(s):
        """Build one-hots for steps [SL*s, SL*(s+1)); boundary col at s%16==0.

        Ordered BEFORE a scan mul in DVE program order so it executes inside
        the mul's semaphore-wait window and never extends the chain.
        """
        c = (SL * s) // NJ
        cb = comb_tiles[c]
        if (SL * s) % NJ == 0:
            nc.vector.tensor_tensor(
                cb[:, 0:T], iotab[:], tprev[:, c:c + 1].broadcast_to([128, T]),
                op=AL.is_equal)
        s0 = (SL * s) % NJ
        oh_view = cb[:, 48:].rearrange("p (j w) -> p j w", w=NB)
        nc.vector.tensor_tensor(
            oh_view[:, s0:s0 + SL, T:NB],
            iorep[:, 0:SL * T].rearrange("p (j w) -> p j w", w=T),
            trc_tiles[c][:, s0 * T:(s0 + SL) * T].rearrange(
                "p (j w) -> p j w", w=T),
            op=AL.is_equal)

    def num_mm(j):
        """Single count/emission matmul for step j (PE, after the scan mm)."""
        c, jj = j // NJ, j % NJ
        cb = comb_tiles[c]
        nc.tensor.matmul(psBig[:], cb[:, NB * jj + 96:NB * jj + 144],
                         cb[:, NB * jj:NB * jj + NB],
                         start=(j == 0), stop=(j == 511), skip_group_check=True)

    # ---------- main scan loop ----------
    # chunk boundaries: small leading chunks so the scan starts early
    bounds = [0, 8, 24, 56] + [56 + 64 * i for i in range(1, 8)] + [512]
    start_of = {}
    for ci in range(len(bounds) - 1):
        for p in range(bounds[ci], bounds[ci + 1]):
            start_of[p] = (ci, bounds[ci], bounds[ci + 1] - bounds[ci])
    expd_tiles = {}
    pend = None  # deferred renorm: (apply_at_k, bcastP)
    nren = 0
    CONS_LAG = 16  # scan steps between one-hot availability and its matmul
    PROD_LEAD = 8  # scan steps a one-hot slice is produced ahead of use
    mm_pending = []
    next_slice = 0
    next_mm = 0
    for k in range(1, NSTEPS + 1):
        ci, c0, clen = start_of[k - 1]
        if k - 1 == c0:
            if ci in raw_tiles:
                raw = raw_tiles[ci]
            else:
                raw = rawp.tile([112, CHUNK * BL], BF16, tag="raw")
                nc.sync.dma_start(raw[0:112, 0:clen * BL],
                                  emstack[:, c0 * BL:(c0 + clen) * BL])
            expd = expdp.tile([112, CHUNK * BL], BF16, tag="expd")
            with tc.tile_wait_until(0.1 + 0.01 * ci):
                nc.scalar.activation(expd[0:112, 0:clen * BL],
                                     raw[0:112, 0:clen * BL], AF.Exp,
                                     bias=cbias[:])
            expd_tiles[ci] = expd
        expd = expd_tiles[ci]
        j = (k - 1) - c0

        # numerator chunk DMA: emit when the previous chunk starts consuming
        # (unstamped: the scheduler hoists it as early as pool buffers allow)
        if (k - CONS_LAG - 1) % NJ == 0 and 1 <= (k - CONS_LAG - 1) // NJ + 1 < NC:
            num_chunk_dma((k - CONS_LAG - 1) // NJ + 1)

        # logical-time stamps pin the per-step interleave: the Tile scheduler
        # otherwise bunches all numerator matmuls into the earliest steps,
        # stretching the latency chain (measured +35us).
        with tc.tile_wait_until(k):
            ps = psp.tile([112, BL], F32, tag="ps")
            nc.tensor.matmul(ps[:], bigm[:], state[:], start=True, stop=True)

        # numerator matmuls ride the PE queue right behind the scan matmul
        # (renorm steps and the bcast-matmul slot two steps later stay clear)
        if k % RK != 0 and (k - 2) % RK != 0:
            budget = 2
            while mm_pending and budget:
                with tc.tile_wait_until(k + 0.25):
                    num_mm(mm_pending.pop(0))
                budget -= 1
        if k - 1 - CONS_LAG >= 0 and next_mm <= k - 1 - CONS_LAG:
            mm_pending.append(next_mm)
            next_mm += 1

        # one-hot production on DVE, ordered BEFORE this step's scan mul
        if k - 1 - CONS_LAG + PROD_LEAD >= SL * next_slice and next_slice < 512 // SL:
            with tc.tile_wait_until(k + 0.4):
                num_slice(next_slice)
            next_slice += 1

        nstate = statep.tile([112, BL], BF16, tag="state")
        eop = expd[:, j * BL:(j + 1) * BL]
        if pend is not None and pend[0] == k:
            with tc.tile_wait_until(k + 0.45):
                esc = expdp.tile([112, BL], BF16, tag="esc")
                nc.vector.tensor_mul(esc[:], eop, pend[1][:])
            eop = esc[:]
            pend = None
        with tc.tile_wait_until(k + 0.5):
            nc.vector.tensor_mul(nstate[:], ps[:], eop)
        state = nstate

        if k % RK == 0 and k < NSTEPS:
            # off-chain: col-sum -> recip -> broadcast; applied at step k+4.
            # Spread over steps k..k+2 so no single DVE/PE window overflows:
            # col-sum mm at k, mx copy on ACT, fast reciprocal in step k+1's
            # DVE window, K=1 broadcast mm in step k+2's numerator slot.
            nren += 1
            with tc.tile_wait_until(k + 0.6):
                mxp = psx.tile([1, BL], F32, tag="mxp")
                nc.tensor.matmul(mxp[:], onescol[:], state[:], start=True, stop=True)
            with tc.tile_wait_until(k + 0.7):
                mx = mstore[0:1, (nren - 1) * BL:nren * BL]
                nc.scalar.activation(mx, mxp[:], AF.Copy)
            with tc.tile_wait_until(k + 1.6):
                rcp = smallp.tile([1, BL], F32, tag="rcp")
                nc.vector.reciprocal_approx_fast(rcp[:], mxp[:])
            with tc.tile_wait_until(k + 2.25):
                bcast = psx.tile([112, BL], F32, tag="bcp")
                nc.tensor.matmul(bcast[:], onesrow[:], rcp[:], start=True, stop=True)
            pend = (k + 4, bcast)

    # leftover numerator work (CONS_LAG tail) overlaps the final combine
    with tc.tile_wait_until(NSTEPS + 1):
        while next_slice < 512 // SL:
            num_slice(next_slice)
            next_slice += 1
        while next_mm <= 511:
            mm_pending.append(next_mm)
            next_mm += 1
        for j2 in mm_pending:
            num_mm(j2)

    # renorm-log pass: mstore is complete after the last renorm (k=448);
    # stamp it into the scan body so the tail doesn't pay the Ln + reduce
    with tc.tile_wait_until(RK * NREN + 8.5):
        lnm = smallp.tile([1, RK * 8], F32, tag="lnm")
        nc.scalar.activation(lnm[:], mstore[:], AF.Ln)
        carry = smallp.tile([1, BL], F32, tag="carry")
        nc.vector.tensor_reduce(
            carry[:], lnm[0:1, :].rearrange("p (j b) -> p b j", j=8), axis=AX.X,
            op=AL.add)

    # ---------- numerator: start/end lookups ----------
    ctx.enter_context(tc.tile_wait_until(NSTEPS + 2))

    def edge_dot(tag_col, vec, name):
        oh0 = smallp.tile([BL, T], BF16, tag=f"oh0{name}")
        i2 = iotab[0:BL, :]
        t2 = tag_col.broadcast_to([BL, T])
        nc.vector.tensor_tensor(oh0[:], i2, t2, op=AL.is_equal)
        cnt = psx.tile([T, 1], F32, tag="cnt")
        nc.tensor.matmul(cnt[:], oh0[:], ones64[:], start=True, stop=True)
        dots = smallp.tile([T, 1], BF16, tag=f"dots{name}")
        with nc.allow_low_precision(reason="scalar total; tolerant"):
            nc.vector.tensor_mul(dots[:], cnt[:], vec)
        ssump = psx.tile([1, 1], F32, tag="sum")
        nc.tensor.matmul(ssump[:], dots[:], ones48[:], start=True, stop=True)
        ssum = smallp.tile([1, 1], F32, tag=f"ssum{name}")
        nc.scalar.activation(ssum[:], ssump[:], AF.Copy)
        return ssum

    stsum = edge_dot(tagS[:, 0:1], stS, "st")
    ensum = edge_dot(tagS[:, S - 1:S], enS, "en")

    # ---------- combine fwd/bwd: Z = a_511 . (M @ g_512) ----------
    psf = psx.tile([T, BL], F32, tag="bcp")
    nc.tensor.matmul(psf[:], bigm[64:112, 64:112], state[64:112, :], start=True,
                     stop=True)
    stateF = smallp.tile([T, BL], F32, tag="stateF")
    nc.scalar.activation(stateF[:], state[0:T, :], AF.Copy)
    z1 = smallp.tile([T, BL], BF16, tag="z1")
    with nc.allow_low_precision(reason="z products; log tolerant"):
        nc.vector.tensor_mul(z1[:], stateF[:], psf[:])
    pz = psx.tile([1, BL], F32, tag="mxp")
    nc.tensor.matmul(pz[:], ones48[:], z1[:], start=True, stop=True)

    lz = smallp.tile([1, BL], F32, tag="lz")
    nc.scalar.activation(lz[:], pz[:], AF.Ln)
    # logZ = lz + 2*carry + 2*NSTEPS*CBIAS
    lzc = smallp.tile([1, BL], F32, tag="lzc")
    nc.vector.scalar_tensor_tensor(lzc[:], carry[:], 2.0, lz[:], op0=AL.mult, op1=AL.add)
    lzsum = smallp.tile([1, 1], F32, tag="lzsum")
    nc.vector.tensor_reduce(lzsum[:], lzc[:], axis=AX.X, op=AL.add)

    # ---------- numerator finalize: sum(psBig * [transitions | I]) ----------
    ct = smallp.tile([T, 2 * T], F32, tag="ct")
    nc.vector.tensor_mul(ct[:], psBig[:], numcoef)
    ctr = smallp.tile([T, 1], F32, tag="ctr")
    nc.vector.tensor_reduce(ctr[:], ct[:], axis=AX.X, op=AL.add)
    ctrb = smallp.tile([T, 1], BF16, tag="ctrb")
    with nc.allow_low_precision(reason="scalar total; tolerant"):
        nc.vector.tensor_copy(ctrb[:], ctr[:])
    ctsump = psx.tile([1, 1], F32, tag="sum")
    nc.tensor.matmul(ctsump[:], ctrb[:], ones48[:], start=True, stop=True)
    ctsum = smallp.tile([1, 1], F32, tag="ctsum")
    nc.scalar.activation(ctsum[:], ctsump[:], AF.Copy)

    # ---------- total = lzsum + finc - ctsum - stsum - ensum ----------
    t1 = smallp.tile([1, 1], F32, tag="t1")
    nc.vector.scalar_tensor_tensor(t1[:], lzsum[:], finc[:], ctsum[:],
                                   op0=AL.add, op1=AL.subtract)
    t2 = smallp.tile([1, 1], F32, tag="t2")
    nc.vector.scalar_tensor_tensor(t2[:], t1[:], stsum[:], ensum[:],
                                   op0=AL.subtract, op1=AL.subtract)
    nc.sync.dma_start(outd[:, :], t2[:])
    ctx.close()


def _prep_core_inputs(em, tags, transitions, start, end):
    """em [BL,S,T] f32, tags [BL,S] int64 -> dict of device arrays."""
    em = np.asarray(em, dtype=np.float32)
    tags = np.asarray(tags).astype(np.int32)

    # emstack [112, 512*BL]: pos j holds slab (j+1): upper em[:,j+1,:]^T,
    # lower em[:,1022-j,:]^T; pos 511 is padding.
    emstack = np.zeros((112, 512, BL), dtype=np.float32)
    emstack[0:T, 0:NSTEPS] = em[:, 1:NSTEPS + 1, :].transpose(2, 1, 0)
    emstack[64:112, 0:NSTEPS] = em[:, S - 2:S - 2 - NSTEPS:-1, :].transpose(2, 1, 0)
    emstack = emstack.reshape(112, 512 * BL).astype(bf16np)

    smallpk = np.zeros((112, SPW), dtype=np.float32)
    smallpk[0:T, 0] = start
    smallpk[64:112, 0] = end
    smallpk[0:T, 1:65] = em[:, 0, :].T
    smallpk[64:112, 1:65] = em[:, S - 1, :].T
    smallpk[0:T, 65:113] = np.ascontiguousarray(transitions.T)
    smallpk[64:112, 65:113] = transitions
    smallpk[0:T, 113] = start
    smallpk[0:T, 114] = end
    smallpk[0:T, 115:115 + T] = transitions
    smallpk[0:T, 115 + T:115 + 2 * T] = np.eye(T, dtype=np.float32)

    # interleaved numerator emissions: per chunk c, 48-col pad then 64 blocks
    # of [em_j (48) | oh_j slot (48, zero)]
    emint = np.zeros((BL * 2, NC, CW), dtype=bf16np)
    emr = em.reshape(BL, 2, NC, NJ, T).reshape(BL * 2, NC, NJ, T)
    emint[:, :, 48:].reshape(BL * 2, NC, NJ, NB)[:, :, :, 0:T] = emr
    emint = emint.reshape(128, NC * CW)

    tpcur = tags.reshape(BL, 2, 512).reshape(BL * 2, 512)
    # prev-tag value for step j=0 of each chunk (sentinel T for (h=0, c=0))
    tprev0 = np.zeros((BL * 2, NC), dtype=np.int32)
    for c in range(NC):
        if c == 0:
            tprev0[0::2, 0] = T
            tprev0[1::2, 0] = tags[:, 511]
        else:
            tprev0[:, c] = tpcur[:, c * NJ - 1]

    iotab = np.tile(np.arange(T, dtype=np.float32), (128, 1))
    iorep = np.tile(np.arange(T, dtype=np.float32), (128, SL))

    return {
        "emstack": emstack,
        "smallpkd": smallpk,
        "emint": emint,
        "tagsnat": tags.astype(bf16np),
        "tpcrep": np.repeat(tpcur.astype(bf16np), T).reshape(128, 512 * T),
        "tprev0": tprev0.astype(bf16np),
        "iotabd": iotab.astype(bf16np),
        "iorepd": iorep.astype(bf16np),
    }


def prep_all_inputs(emissions, tags, mask, transitions, start_transitions,
                    end_transitions):
    em = np.asarray(emissions, dtype=np.float32)
    tg = np.asarray(tags)
    tr = np.asarray(transitions, dtype=np.float32)
    st = np.asarray(start_transitions, dtype=np.float32)
    en = np.asarray(end_transitions, dtype=np.float32)
    return [
        _prep_core_inputs(em[c * BL:(c + 1) * BL], tg[c * BL:(c + 1) * BL], tr, st, en)
        for c in range(NCORES)
    ]


_NC_CACHE = {}


def get_graph():
    if "nc" not in _NC_CACHE:
        _NC_CACHE["nc"] = _build_graph()
    return _NC_CACHE["nc"]


def kernel(emissions, tags, mask, transitions, start_transitions, end_transitions,
           **kw):
    from concourse import bass_utils
    nc = get_graph()
    in_maps = prep_all_inputs(emissions, tags, mask, transitions,
                              start_transitions, end_transitions)
    res = bass_utils.run_bass_kernel_spmd(nc, in_maps, core_ids=list(range(NCORES)))
    total = sum(float(res.results[c]["out"][0, 0]) for c in range(NCORES))
    return np.float32(total / B)


if __name__ == "__main__":
    get_graph()
    print("graph built ok")
